# revision 8
# baseline (speedup 1.0000x reference)
"""DeepSeekMoE forward on 8 TRN2 NeuronCores.

Strategy (expert-parallel, per the sharding hint):
  - Host computes the (tiny) gate: scores = sqrt(softplus(x @ gate_w)),
    top-2 selection, normalized combine weights, and builds per-expert
    token lists (the "all-to-all dispatch" done host-side since kernel()
    receives full inputs and returns the full output).
  - Core e holds routed expert e's weights and processes the tokens
    routed to it (padded to a common capacity C).
  - The shared expert is split along its intermediate dim I across the
    8 cores (each core computes a 384-wide slice for ALL tokens); the
    partial outputs sum to the exact shared-expert output.
  - Host scatters/sums the per-core outputs back to [B, T, D].

Device compute is bf16 (f32 PSUM accumulation): TRN2 PE does bf16 at
1 cycle/row vs 4 for fp32, and bf16 halves the HBM traffic.
"""

import math

import numpy as np
import ml_dtypes

import concourse.bass as bass
import concourse.tile as tile
from concourse import bacc, mybir
from concourse.bass_utils import run_bass_kernel_spmd

BF16 = np.dtype(ml_dtypes.bfloat16)
DT_BF16 = mybir.dt.bfloat16
DT_F32 = mybir.dt.float32

D = 768            # n_embd
I = 3072           # moe_intermediate_size
E = 8              # n_routed_experts
TOPK = 2
LIMIT = 10.0
NTOK = 2048        # B*T
NCORES = 8
ISH = I // NCORES  # shared-expert I slice per core (384)
DTILES = D // 128  # 6
MI = I // 128      # 24 routed i-tiles
MS = ISH // 128    # 3 shared i-tiles

_BUILD_CACHE: dict = {}
last_results = None  # BassKernelResults of the most recent run (for test.py)


def _chunks(total, step=512):
    out = []
    t0 = 0
    while t0 < total:
        out.append((t0, min(step, total - t0)))
        t0 += step
    return out


def _build(C):
    """Build the SPMD Bass graph for capacity C (tokens per routed expert)."""
    nc = bacc.Bacc("TRN2", target_bir_lowering=False, debug=False)

    ap = lambda name, shape, dt, kind: nc.dram_tensor(name, shape, dt, kind=kind).ap()
    w13 = ap("w13", [2 * MI, 128, DTILES, 128], DT_BF16, "ExternalInput")
    w2 = ap("w2", [128, MI, D], DT_BF16, "ExternalInput")
    w13s = ap("w13s", [2 * MS, 128, DTILES, 128], DT_BF16, "ExternalInput")
    w2s = ap("w2s", [128, MS, D], DT_BF16, "ExternalInput")
    xt = ap("xt", [128, DTILES, NTOK], DT_BF16, "ExternalInput")
    xg = ap("xg", [128, DTILES, C], DT_BF16, "ExternalInput")
    cw = ap("cw", [128, C // 128], DT_F32, "ExternalInput")
    out_r = ap("out_r", [C, D], DT_F32, "ExternalOutput")
    out_s = ap("out_s", [NTOK, D], DT_F32, "ExternalOutput")

    TCR = _chunks(C)      # routed token chunks
    TCS = _chunks(NTOK)   # shared token chunks
    DC = _chunks(D)       # output d chunks (512, 256)

    MIN = mybir.AluOpType.min
    MAX = mybir.AluOpType.max
    SILU = mybir.ActivationFunctionType.Silu
    COPY = mybir.ActivationFunctionType.Copy

    with tile.TileContext(nc) as tc:
        with (
            tc.tile_pool(name="res", bufs=1) as res,
            tc.tile_pool(name="slab", bufs=6) as slabs,
            tc.tile_pool(name="tmp", bufs=4) as tmps,
            tc.tile_pool(name="ev", bufs=4) as evs,
            tc.tile_pool(name="ps", bufs=8, space="PSUM") as ps1,
        ):
            ps2 = ps1
            # xg first: it gates the very first matmul. The other resident
            # tensors are needed only by later phases — their DMAs are
            # issued mid-way through the GEMM1 loop (side_loads) so the
            # startup slab stream gets the full HBM bandwidth.
            xg_sb = res.tile([128, DTILES, C], DT_BF16)
            nc.sync.dma_start(xg_sb[:], xg[:])
            xt_sb = res.tile([128, DTILES, NTOK], DT_BF16)
            w2_sb = res.tile([128, MI, D], DT_BF16)
            w2s_sb = res.tile([128, MS, D], DT_BF16)
            cw_sb = res.tile([128, C // 128], DT_F32)
            h_sb = res.tile([128, MI, C], DT_BF16)
            hs_sb = res.tile([128, MS, NTOK], DT_BF16)

            def gemm1(npairs, wsrc, x_sb, tchunks, hout, side_loads={}):
                # hout[i, t] = silu(min(W1.T x, L)) * clip(W3.T x, -L, L)
                for m in range(npairs):
                    for fn in side_loads.get(m, []):
                        fn()
                    sg = slabs.tile([128, DTILES, 128], DT_BF16, tag="slab")
                    nc.sync.dma_start(sg[:], wsrc[2 * m])
                    su = slabs.tile([128, DTILES, 128], DT_BF16, tag="slab")
                    nc.sync.dma_start(su[:], wsrc[2 * m + 1])
                    for (t0, tl) in tchunks:
                        pg = ps1.tile([128, 512], DT_F32, tag="ps", name="pg")[:, :tl]
                        pu = ps1.tile([128, 512], DT_F32, tag="ps", name="pu")[:, :tl]
                        for d in range(DTILES):
                            nc.tensor.matmul(
                                pg[:], sg[:, d, :], x_sb[:, d, t0:t0 + tl],
                                start=(d == 0), stop=(d == DTILES - 1))
                        for d in range(DTILES):
                            nc.tensor.matmul(
                                pu[:], su[:, d, :], x_sb[:, d, t0:t0 + tl],
                                start=(d == 0), stop=(d == DTILES - 1))
                        tg = tmps.tile([128, 512], DT_F32, tag="tg", name="tg")[:, :tl]
                        nc.vector.tensor_scalar(tg[:], pg[:], LIMIT, None, MIN)
                        sa = tmps.tile([128, 512], DT_F32, tag="sa", name="sa")[:, :tl]
                        nc.scalar.activation(sa[:], tg[:], SILU)
                        tu = tmps.tile([128, 512], DT_F32, tag="tu", name="tu")[:, :tl]
                        nc.vector.tensor_scalar(tu[:], pu[:], LIMIT, -LIMIT, MIN, MAX)
                        nc.vector.tensor_mul(hout[:, m, t0:t0 + tl], sa[:], tu[:])

            def gemm2(nitiles, h, w2sb, ttiles, scale_col, dst):
                # dst[t, d] = (h.T @ w2) * cw[t]
                for tt in range(ttiles):
                    for (d0, dl) in DC:
                        ps = ps2.tile([128, 512], DT_F32, tag="ps", name="po")[:, :dl]
                        for m in range(nitiles):
                            nc.tensor.matmul(
                                ps[:], h[:, m, tt * 128:(tt + 1) * 128],
                                w2sb[:, m, d0:d0 + dl],
                                start=(m == 0), stop=(m == nitiles - 1))
                        ev = evs.tile([128, 512], DT_F32, tag="ev", name="ev")[:, :dl]
                        if scale_col is not None:
                            nc.scalar.activation(ev[:], ps[:], COPY,
                                                 scale=scale_col[:, tt:tt + 1])
                        else:
                            nc.scalar.activation(ev[:], ps[:], COPY)
                        nc.sync.dma_start(dst[tt * 128:(tt + 1) * 128, d0:d0 + dl], ev[:])

            side = {
                4: [lambda: nc.gpsimd.dma_start(xt_sb[:], xt[:])],
                10: [lambda: nc.gpsimd.dma_start(w2_sb[:], w2[:])],
                16: [lambda: nc.gpsimd.dma_start(w2s_sb[:], w2s[:]),
                     lambda: nc.gpsimd.dma_start(cw_sb[:], cw[:])],
            }
            gemm1(MI, w13, xg_sb, TCR, h_sb, side)
            gemm1(MS, w13s, xt_sb, TCS, hs_sb)
            gemm2(MI, h_sb, w2_sb, C // 128, cw_sb, out_r)
            gemm2(MS, hs_sb, w2s_sb, NTOK // 128, None, out_s)

    nc.compile()
    return nc


def _slabify(w):
    """[768, ncols] -> [ncols//128, 128, 6, 128] stationary slabs.

    slab[m, p, a, f] = w[a*128 + p, m*128 + f]
    """
    ncols = w.shape[1]
    return np.ascontiguousarray(
        w.reshape(DTILES, 128, ncols // 128, 128).transpose(2, 1, 0, 3))


def _ptile(a):
    """[R, cols] with R = n*128 -> [128, n, cols] (partition-major)."""
    r, c = a.shape
    return np.ascontiguousarray(a.reshape(r // 128, 128, c).transpose(1, 0, 2))


def kernel(**inputs) -> np.ndarray:
    global last_results
    x = np.asarray(inputs["x"], dtype=np.float32)
    gate_w = np.asarray(inputs["gate_w"], dtype=np.float32)
    gate_bias = np.asarray(inputs["gate_bias"], dtype=np.float32)
    w1 = np.asarray(inputs["w1"], dtype=np.float32)
    w2 = np.asarray(inputs["w2"], dtype=np.float32)
    w3 = np.asarray(inputs["w3"], dtype=np.float32)
    w1s = np.asarray(inputs["w1s"], dtype=np.float32)
    w2s = np.asarray(inputs["w2s"], dtype=np.float32)
    w3s = np.asarray(inputs["w3s"], dtype=np.float32)

    B, T, _ = x.shape
    N = B * T
    assert N == NTOK, f"kernel compiled for {NTOK} tokens, got {N}"
    flat = x.reshape(N, D)

    # ---- gate (host, f32, mirrors reference semantics) ----
    logits = flat @ gate_w                              # [N, E]
    scores = np.sqrt(np.logaddexp(np.float32(0.0), logits)).astype(np.float32)
    routed = scores + gate_bias
    idx = np.argsort(-routed, axis=1, kind="stable")[:, :TOPK]      # [N, K]
    wts = np.take_along_axis(scores, idx, axis=1)
    wts = wts / np.clip(wts.sum(axis=1, keepdims=True), 1e-6, None)

    # ---- dispatch: per-expert token lists ----
    ee = idx.reshape(-1)
    tok = np.repeat(np.arange(N), TOPK)
    ww = wts.reshape(-1).astype(np.float32)
    toks, cwts, counts = [], [], []
    for e in range(E):
        sel = ee == e
        toks.append(tok[sel])
        cwts.append(ww[sel])
        counts.append(int(sel.sum()))
    C = max(128, ((max(counts) + 127) // 128) * 128)

    # ---- per-core input maps ----
    xt_h = _ptile(flat.T.astype(BF16))                  # [128, 6, N]
    in_maps = []
    for e in range(E):
        ce = counts[e]
        xg_full = np.zeros((C, D), dtype=np.float32)
        xg_full[:ce] = flat[toks[e]]
        cw_full = np.zeros(C, dtype=np.float32)
        cw_full[:ce] = cwts[e]

        w13 = np.empty((2 * MI, 128, DTILES, 128), dtype=BF16)
        w13[0::2] = _slabify(w1[e].astype(BF16))
        w13[1::2] = _slabify(w3[e].astype(BF16))
        sl = slice(e * ISH, (e + 1) * ISH)
        w13s = np.empty((2 * MS, 128, DTILES, 128), dtype=BF16)
        w13s[0::2] = _slabify(w1s[:, sl].astype(BF16))
        w13s[1::2] = _slabify(w3s[:, sl].astype(BF16))

        in_maps.append({
            "w13": w13,
            "w2": _ptile(w2[e].astype(BF16)),           # [128, 24, 768]
            "w13s": w13s,
            "w2s": _ptile(w2s[sl].astype(BF16)),        # [128, 3, 768]
            "xt": xt_h,
            "xg": _ptile(xg_full.T.astype(BF16)),       # [128, 6, C]
            "cw": np.ascontiguousarray(
                cw_full.reshape(C // 128, 128).T),      # [128, C//128]
        })

    # ---- build + run ----
    if C not in _BUILD_CACHE:
        _BUILD_CACHE[C] = _build(C)
    nc = _BUILD_CACHE[C]
    last_results = run_bass_kernel_spmd(nc, in_maps, core_ids=list(range(NCORES)))
    res = last_results.results

    # ---- combine (host): sum shared partials, scatter routed outputs ----
    out = res[0]["out_s"].astype(np.float32).copy()
    for c in range(1, NCORES):
        out += res[c]["out_s"]
    for e in range(E):
        ce = counts[e]
        if ce:
            out[toks[e]] += res[e]["out_r"][:ce]
    return out.reshape(B, T, D).astype(np.float32)


# revision 9
# speedup vs baseline: 1.0977x; 1.0977x over previous
"""DeepSeekMoE forward on 8 TRN2 NeuronCores.

Strategy (expert-parallel, per the sharding hint):
  - Host computes the (tiny) gate: scores = sqrt(softplus(x @ gate_w)),
    top-2 selection, normalized combine weights, and builds per-expert
    token lists (the "all-to-all dispatch" done host-side since kernel()
    receives full inputs and returns the full output).
  - Core e holds routed expert e's weights and processes the tokens
    routed to it (padded to a common capacity C).
  - The shared expert is split along its intermediate dim I across the
    8 cores (each core computes a 384-wide slice for ALL tokens); the
    partial outputs sum to the exact shared-expert output.
  - Host scatters/sums the per-core outputs back to [B, T, D].

Device compute is bf16 (f32 PSUM accumulation): TRN2 PE does bf16 at
1 cycle/row vs 4 for fp32, and bf16 halves the HBM traffic.
"""

import math

import numpy as np
import ml_dtypes

import concourse.bass as bass
import concourse.tile as tile
from concourse import bacc, mybir
from concourse.bass_utils import run_bass_kernel_spmd

BF16 = np.dtype(ml_dtypes.bfloat16)
DT_BF16 = mybir.dt.bfloat16
DT_F32 = mybir.dt.float32

D = 768            # n_embd
I = 3072           # moe_intermediate_size
E = 8              # n_routed_experts
TOPK = 2
LIMIT = 10.0
NTOK = 2048        # B*T
NCORES = 8
ISH = I // NCORES  # shared-expert I slice per core (384)
DTILES = D // 128  # 6
MI = I // 128      # 24 routed i-tiles
MS = ISH // 128    # 3 shared i-tiles

_BUILD_CACHE: dict = {}
last_results = None  # BassKernelResults of the most recent run (for test.py)


def _chunks(total, step=512):
    out = []
    t0 = 0
    while t0 < total:
        out.append((t0, min(step, total - t0)))
        t0 += step
    return out


def _build(C):
    """Build the SPMD Bass graph for capacity C (tokens per routed expert)."""
    nc = bacc.Bacc("TRN2", target_bir_lowering=False, debug=False)

    ap = lambda name, shape, dt, kind: nc.dram_tensor(name, shape, dt, kind=kind).ap()
    w13 = ap("w13", [2 * MI, 128, DTILES, 128], DT_BF16, "ExternalInput")
    w2 = ap("w2", [128, MI, D], DT_BF16, "ExternalInput")
    w13s = ap("w13s", [2 * MS, 128, DTILES, 128], DT_BF16, "ExternalInput")
    w2s = ap("w2s", [128, MS, D], DT_BF16, "ExternalInput")
    xt = ap("xt", [128, DTILES, NTOK], DT_BF16, "ExternalInput")
    xg = ap("xg", [128, DTILES, C], DT_BF16, "ExternalInput")
    cw = ap("cw", [128, C // 128], DT_F32, "ExternalInput")
    out_r = ap("out_r", [C, D], DT_F32, "ExternalOutput")
    out_s = ap("out_s", [NTOK, D], DT_F32, "ExternalOutput")

    TCR = _chunks(C)      # routed token chunks
    TCS = _chunks(NTOK)   # shared token chunks
    DC = _chunks(D)       # output d chunks (512, 256)

    MIN = mybir.AluOpType.min
    MAX = mybir.AluOpType.max
    SILU = mybir.ActivationFunctionType.Silu
    COPY = mybir.ActivationFunctionType.Copy

    with tile.TileContext(nc) as tc:
        with (
            tc.tile_pool(name="res", bufs=1) as res,
            tc.tile_pool(name="slab", bufs=6) as slabs,
            tc.tile_pool(name="tmp", bufs=4) as tmps,
            tc.tile_pool(name="ev", bufs=4) as evs,
            tc.tile_pool(name="ps", bufs=8, space="PSUM") as ps1,
        ):
            ps2 = ps1
            # xg first: it gates the very first matmul. The other resident
            # tensors are needed only by later phases — their DMAs are
            # issued mid-way through the GEMM1 loop (side_loads) so the
            # startup slab stream gets the full HBM bandwidth.
            xg_sb = res.tile([128, DTILES, C], DT_BF16)
            nc.sync.dma_start(xg_sb[:], xg[:])
            xt_sb = res.tile([128, DTILES, NTOK], DT_BF16)
            w2_sb = res.tile([128, MI, D], DT_BF16)
            w2s_sb = res.tile([128, MS, D], DT_BF16)
            cw_sb = res.tile([128, C // 128], DT_F32)
            h_sb = res.tile([128, MI, C], DT_BF16)
            hs_sb = res.tile([128, MS, NTOK], DT_BF16)

            def gemm1(npairs, wsrc, x_sb, tchunks, hout, side_loads={}):
                # hout[i, t] = silu(min(W1.T x, L)) * clip(W3.T x, -L, L)
                for m in range(npairs):
                    for fn in side_loads.get(m, []):
                        fn()
                    sg = slabs.tile([128, DTILES, 128], DT_BF16, tag="slab")
                    nc.sync.dma_start(sg[:], wsrc[2 * m])
                    su = slabs.tile([128, DTILES, 128], DT_BF16, tag="slab")
                    nc.sync.dma_start(su[:], wsrc[2 * m + 1])
                    for (t0, tl) in tchunks:
                        pg = ps1.tile([128, 512], DT_F32, tag="ps", name="pg")[:, :tl]
                        pu = ps1.tile([128, 512], DT_F32, tag="ps", name="pu")[:, :tl]
                        for d in range(DTILES):
                            nc.tensor.matmul(
                                pg[:], sg[:, d, :], x_sb[:, d, t0:t0 + tl],
                                start=(d == 0), stop=(d == DTILES - 1))
                        for d in range(DTILES):
                            nc.tensor.matmul(
                                pu[:], su[:, d, :], x_sb[:, d, t0:t0 + tl],
                                start=(d == 0), stop=(d == DTILES - 1))
                        tg = tmps.tile([128, 512], DT_F32, tag="tg", name="tg")[:, :tl]
                        nc.vector.tensor_scalar(tg[:], pg[:], LIMIT, None, MIN)
                        sa = tmps.tile([128, 512], DT_F32, tag="sa", name="sa")[:, :tl]
                        nc.scalar.activation(sa[:], tg[:], SILU)
                        tu = tmps.tile([128, 512], DT_F32, tag="tu", name="tu")[:, :tl]
                        nc.vector.tensor_scalar(tu[:], pu[:], LIMIT, -LIMIT, MIN, MAX)
                        nc.vector.tensor_mul(hout[:, m, t0:t0 + tl], sa[:], tu[:])

            def gemm2(nitiles, h, w2sb, ttiles, scale_col, dst):
                # dst[t, d] = (h.T @ w2) * cw[t]
                for tt in range(ttiles):
                    for (d0, dl) in DC:
                        ps = ps2.tile([128, 512], DT_F32, tag="ps", name="po")[:, :dl]
                        for m in range(nitiles):
                            nc.tensor.matmul(
                                ps[:], h[:, m, tt * 128:(tt + 1) * 128],
                                w2sb[:, m, d0:d0 + dl],
                                start=(m == 0), stop=(m == nitiles - 1))
                        ev = evs.tile([128, 512], DT_F32, tag="ev", name="ev")[:, :dl]
                        if scale_col is not None:
                            nc.scalar.activation(ev[:], ps[:], COPY,
                                                 scale=scale_col[:, tt:tt + 1])
                        else:
                            nc.scalar.activation(ev[:], ps[:], COPY)
                        nc.sync.dma_start(dst[tt * 128:(tt + 1) * 128, d0:d0 + dl], ev[:])

            # Interleave the later-phase resident loads into the slab DMA
            # FIFO in small chunks so they never starve the slab stream.
            side = {}
            for j, d in enumerate(range(DTILES)):
                side.setdefault(4 + 2 * j, []).append(
                    lambda d=d: nc.sync.dma_start(xt_sb[:, d, :], xt[:, d, :]))
            for j in range(4):
                side.setdefault(16 + 2 * j, []).append(
                    lambda j=j: nc.sync.dma_start(
                        w2_sb[:, 6 * j:6 * (j + 1), :], w2[:, 6 * j:6 * (j + 1), :]))
            side.setdefault(11, []).append(
                lambda: nc.sync.dma_start(cw_sb[:], cw[:]))
            side.setdefault(13, []).append(
                lambda: nc.sync.dma_start(w2s_sb[:], w2s[:]))
            gemm1(MI, w13, xg_sb, TCR, h_sb, side)
            gemm1(MS, w13s, xt_sb, TCS, hs_sb)
            gemm2(MI, h_sb, w2_sb, C // 128, cw_sb, out_r)
            gemm2(MS, hs_sb, w2s_sb, NTOK // 128, None, out_s)

    nc.compile()
    return nc


def _slabify(w):
    """[768, ncols] -> [ncols//128, 128, 6, 128] stationary slabs.

    slab[m, p, a, f] = w[a*128 + p, m*128 + f]
    """
    ncols = w.shape[1]
    return np.ascontiguousarray(
        w.reshape(DTILES, 128, ncols // 128, 128).transpose(2, 1, 0, 3))


def _ptile(a):
    """[R, cols] with R = n*128 -> [128, n, cols] (partition-major)."""
    r, c = a.shape
    return np.ascontiguousarray(a.reshape(r // 128, 128, c).transpose(1, 0, 2))


def kernel(**inputs) -> np.ndarray:
    global last_results
    x = np.asarray(inputs["x"], dtype=np.float32)
    gate_w = np.asarray(inputs["gate_w"], dtype=np.float32)
    gate_bias = np.asarray(inputs["gate_bias"], dtype=np.float32)
    w1 = np.asarray(inputs["w1"], dtype=np.float32)
    w2 = np.asarray(inputs["w2"], dtype=np.float32)
    w3 = np.asarray(inputs["w3"], dtype=np.float32)
    w1s = np.asarray(inputs["w1s"], dtype=np.float32)
    w2s = np.asarray(inputs["w2s"], dtype=np.float32)
    w3s = np.asarray(inputs["w3s"], dtype=np.float32)

    B, T, _ = x.shape
    N = B * T
    assert N == NTOK, f"kernel compiled for {NTOK} tokens, got {N}"
    flat = x.reshape(N, D)

    # ---- gate (host, f32, mirrors reference semantics) ----
    logits = flat @ gate_w                              # [N, E]
    scores = np.sqrt(np.logaddexp(np.float32(0.0), logits)).astype(np.float32)
    routed = scores + gate_bias
    idx = np.argsort(-routed, axis=1, kind="stable")[:, :TOPK]      # [N, K]
    wts = np.take_along_axis(scores, idx, axis=1)
    wts = wts / np.clip(wts.sum(axis=1, keepdims=True), 1e-6, None)

    # ---- dispatch: per-expert token lists ----
    ee = idx.reshape(-1)
    tok = np.repeat(np.arange(N), TOPK)
    ww = wts.reshape(-1).astype(np.float32)
    toks, cwts, counts = [], [], []
    for e in range(E):
        sel = ee == e
        toks.append(tok[sel])
        cwts.append(ww[sel])
        counts.append(int(sel.sum()))
    C = max(128, ((max(counts) + 127) // 128) * 128)

    # ---- per-core input maps ----
    xt_h = _ptile(flat.T.astype(BF16))                  # [128, 6, N]
    in_maps = []
    for e in range(E):
        ce = counts[e]
        xg_full = np.zeros((C, D), dtype=np.float32)
        xg_full[:ce] = flat[toks[e]]
        cw_full = np.zeros(C, dtype=np.float32)
        cw_full[:ce] = cwts[e]

        w13 = np.empty((2 * MI, 128, DTILES, 128), dtype=BF16)
        w13[0::2] = _slabify(w1[e].astype(BF16))
        w13[1::2] = _slabify(w3[e].astype(BF16))
        sl = slice(e * ISH, (e + 1) * ISH)
        w13s = np.empty((2 * MS, 128, DTILES, 128), dtype=BF16)
        w13s[0::2] = _slabify(w1s[:, sl].astype(BF16))
        w13s[1::2] = _slabify(w3s[:, sl].astype(BF16))

        in_maps.append({
            "w13": w13,
            "w2": _ptile(w2[e].astype(BF16)),           # [128, 24, 768]
            "w13s": w13s,
            "w2s": _ptile(w2s[sl].astype(BF16)),        # [128, 3, 768]
            "xt": xt_h,
            "xg": _ptile(xg_full.T.astype(BF16)),       # [128, 6, C]
            "cw": np.ascontiguousarray(
                cw_full.reshape(C // 128, 128).T),      # [128, C//128]
        })

    # ---- build + run ----
    if C not in _BUILD_CACHE:
        _BUILD_CACHE[C] = _build(C)
    nc = _BUILD_CACHE[C]
    last_results = run_bass_kernel_spmd(nc, in_maps, core_ids=list(range(NCORES)))
    res = last_results.results

    # ---- combine (host): sum shared partials, scatter routed outputs ----
    out = res[0]["out_s"].astype(np.float32).copy()
    for c in range(1, NCORES):
        out += res[c]["out_s"]
    for e in range(E):
        ce = counts[e]
        if ce:
            out[toks[e]] += res[e]["out_r"][:ce]
    return out.reshape(B, T, D).astype(np.float32)


# revision 12
# speedup vs baseline: 1.1382x; 1.0369x over previous
"""DeepSeekMoE forward on 8 TRN2 NeuronCores.

Strategy (expert-parallel, per the sharding hint):
  - Host computes the (tiny) gate: scores = sqrt(softplus(x @ gate_w)),
    top-2 selection, normalized combine weights, and builds per-expert
    token lists (the "all-to-all dispatch" done host-side since kernel()
    receives full inputs and returns the full output).
  - Core e holds routed expert e's weights and processes the tokens
    routed to it (padded to a common capacity C).
  - The shared expert is split along its intermediate dim I across the
    8 cores (each core computes a 384-wide slice for ALL tokens); the
    partial outputs sum to the exact shared-expert output.
  - Host scatters/sums the per-core outputs back to [B, T, D].

Device compute is bf16 (f32 PSUM accumulation): TRN2 PE does bf16 at
1 cycle/row vs 4 for fp32, and bf16 halves the HBM traffic.
"""

import math

import numpy as np
import ml_dtypes

import concourse.bass as bass
import concourse.tile as tile
from concourse import bacc, mybir
from concourse.bass_utils import run_bass_kernel_spmd

BF16 = np.dtype(ml_dtypes.bfloat16)
DT_BF16 = mybir.dt.bfloat16
DT_F32 = mybir.dt.float32

D = 768            # n_embd
I = 3072           # moe_intermediate_size
E = 8              # n_routed_experts
TOPK = 2
LIMIT = 10.0
NTOK = 2048        # B*T
NCORES = 8
ISH = I // NCORES  # shared-expert I slice per core (384)
DTILES = D // 128  # 6
MI = I // 128      # 24 routed i-tiles
MS = ISH // 128    # 3 shared i-tiles

_BUILD_CACHE: dict = {}
last_results = None  # BassKernelResults of the most recent run (for test.py)


def _chunks(total, step=512):
    out = []
    t0 = 0
    while t0 < total:
        out.append((t0, min(step, total - t0)))
        t0 += step
    return out


def _build(C):
    """Build the SPMD Bass graph for capacity C (tokens per routed expert)."""
    nc = bacc.Bacc("TRN2", target_bir_lowering=False, debug=False)

    ap = lambda name, shape, dt, kind: nc.dram_tensor(name, shape, dt, kind=kind).ap()
    w13 = ap("w13", [2 * MI, 128, DTILES, 128], DT_BF16, "ExternalInput")
    w2 = ap("w2", [128, MI, D], DT_BF16, "ExternalInput")
    w13s = ap("w13s", [2 * MS, 128, DTILES, 128], DT_BF16, "ExternalInput")
    w2s = ap("w2s", [128, MS, D], DT_BF16, "ExternalInput")
    xt = ap("xt", [128, DTILES, NTOK], DT_BF16, "ExternalInput")
    xg = ap("xg", [128, DTILES, C], DT_BF16, "ExternalInput")
    cw = ap("cw", [128, (C + 127) // 128], DT_F32, "ExternalInput")
    out_r = ap("out_r", [C, D], DT_F32, "ExternalOutput")
    out_s = ap("out_s", [NTOK, D], DT_F32, "ExternalOutput")

    TCR = _chunks(C)      # routed token chunks
    TCS = _chunks(NTOK)   # shared token chunks
    DC = _chunks(D)       # output d chunks (512, 256)

    MIN = mybir.AluOpType.min
    MAX = mybir.AluOpType.max
    SILU = mybir.ActivationFunctionType.Silu
    COPY = mybir.ActivationFunctionType.Copy

    with tile.TileContext(nc) as tc:
        with (
            tc.tile_pool(name="res", bufs=1) as res,
            tc.tile_pool(name="slab", bufs=6) as slabs,
            tc.tile_pool(name="tmp", bufs=4) as tmps,
            tc.tile_pool(name="ev", bufs=4) as evs,
            tc.tile_pool(name="ps", bufs=8, space="PSUM") as ps1,
        ):
            ps2 = ps1
            # xg first: it gates the very first matmul. The other resident
            # tensors are needed only by later phases — their DMAs are
            # issued mid-way through the GEMM1 loop (side_loads) so the
            # startup slab stream gets the full HBM bandwidth.
            xg_sb = res.tile([128, DTILES, C], DT_BF16)
            for d in range(DTILES):
                nc.sync.dma_start(xg_sb[:, d, :], xg[:, d, :])
            xt_sb = res.tile([128, DTILES, NTOK], DT_BF16)
            w2_sb = res.tile([128, MI, D], DT_BF16)
            w2s_sb = res.tile([128, MS, D], DT_BF16)
            cw_sb = res.tile([128, (C + 127) // 128], DT_F32)
            h_sb = res.tile([128, MI, C], DT_BF16)
            hs_sb = res.tile([128, MS, NTOK], DT_BF16)

            def gemm1(npairs, wsrc, x_sb, tchunks, hout, side_loads={}):
                # hout[i, t] = silu(min(W1.T x, L)) * clip(W3.T x, -L, L)
                for m in range(npairs):
                    for fn in side_loads.get(m, []):
                        fn()
                    sg = slabs.tile([128, DTILES, 128], DT_BF16, tag="slab")
                    nc.sync.dma_start(sg[:], wsrc[2 * m])
                    su = slabs.tile([128, DTILES, 128], DT_BF16, tag="slab")
                    nc.sync.dma_start(su[:], wsrc[2 * m + 1])
                    for (t0, tl) in tchunks:
                        pg = ps1.tile([128, 512], DT_F32, tag="ps", name="pg")[:, :tl]
                        pu = ps1.tile([128, 512], DT_F32, tag="ps", name="pu")[:, :tl]
                        for d in range(DTILES):
                            nc.tensor.matmul(
                                pg[:], sg[:, d, :], x_sb[:, d, t0:t0 + tl],
                                start=(d == 0), stop=(d == DTILES - 1))
                        for d in range(DTILES):
                            nc.tensor.matmul(
                                pu[:], su[:, d, :], x_sb[:, d, t0:t0 + tl],
                                start=(d == 0), stop=(d == DTILES - 1))
                        tg = tmps.tile([128, 512], DT_F32, tag="tg", name="tg")[:, :tl]
                        nc.vector.tensor_scalar(tg[:], pg[:], LIMIT, None, MIN)
                        sa = tmps.tile([128, 512], DT_F32, tag="sa", name="sa")[:, :tl]
                        nc.scalar.activation(sa[:], tg[:], SILU)
                        tu = tmps.tile([128, 512], DT_F32, tag="tu", name="tu")[:, :tl]
                        nc.vector.tensor_scalar(tu[:], pu[:], LIMIT, -LIMIT, MIN, MAX)
                        nc.vector.tensor_mul(hout[:, m, t0:t0 + tl], sa[:], tu[:])

            def gemm2(nitiles, h, w2sb, tlen_total, scale_col, dst, alt_evict=False):
                # dst[t, d] = (h.T @ w2) * cw[t]
                for tt, (t0, tl) in enumerate(_chunks(tlen_total, 128)):
                    for di, (d0, dl) in enumerate(DC):
                        ps = ps2.tile([128, 512], DT_F32, tag="ps", name="po")[:tl, :dl]
                        for m in range(nitiles):
                            nc.tensor.matmul(
                                ps[:], h[:, m, t0:t0 + tl],
                                w2sb[:, m, d0:d0 + dl],
                                start=(m == 0), stop=(m == nitiles - 1))
                        ev = evs.tile([128, 512], DT_F32, tag="ev", name="ev")[:tl, :dl]
                        if scale_col is not None:
                            nc.scalar.activation(ev[:], ps[:], COPY,
                                                 scale=scale_col[:tl, tt:tt + 1])
                        elif alt_evict and di % 2 == 0:
                            nc.vector.tensor_copy(ev[:], ps[:])
                        else:
                            nc.scalar.activation(ev[:], ps[:], COPY)
                        nc.sync.dma_start(dst[t0:t0 + tl, d0:d0 + dl], ev[:])

            # Interleave the later-phase resident loads into the slab DMA
            # FIFO in small chunks so they never starve the slab stream.
            side = {}
            for j, d in enumerate(range(DTILES)):
                side.setdefault(4 + 2 * j, []).append(
                    lambda d=d: nc.sync.dma_start(xt_sb[:, d, :], xt[:, d, :]))
            for j in range(4):
                side.setdefault(16 + 2 * j, []).append(
                    lambda j=j: nc.sync.dma_start(
                        w2_sb[:, 6 * j:6 * (j + 1), :], w2[:, 6 * j:6 * (j + 1), :]))
            side.setdefault(11, []).append(
                lambda: nc.sync.dma_start(cw_sb[:], cw[:]))
            side.setdefault(13, []).append(
                lambda: nc.sync.dma_start(w2s_sb[:], w2s[:]))
            gemm1(MI, w13, xg_sb, TCR, h_sb, side)
            gemm1(MS, w13s, xt_sb, TCS, hs_sb)
            gemm2(MI, h_sb, w2_sb, C, cw_sb, out_r)
            gemm2(MS, hs_sb, w2s_sb, NTOK, None, out_s, alt_evict=True)

    nc.compile()
    return nc


def _slabify(w):
    """[768, ncols] -> [ncols//128, 128, 6, 128] stationary slabs.

    slab[m, p, a, f] = w[a*128 + p, m*128 + f]
    """
    ncols = w.shape[1]
    return np.ascontiguousarray(
        w.reshape(DTILES, 128, ncols // 128, 128).transpose(2, 1, 0, 3))


def _ptile(a):
    """[R, cols] with R = n*128 -> [128, n, cols] (partition-major)."""
    r, c = a.shape
    return np.ascontiguousarray(a.reshape(r // 128, 128, c).transpose(1, 0, 2))


def kernel(**inputs) -> np.ndarray:
    global last_results
    x = np.asarray(inputs["x"], dtype=np.float32)
    gate_w = np.asarray(inputs["gate_w"], dtype=np.float32)
    gate_bias = np.asarray(inputs["gate_bias"], dtype=np.float32)
    w1 = np.asarray(inputs["w1"], dtype=np.float32)
    w2 = np.asarray(inputs["w2"], dtype=np.float32)
    w3 = np.asarray(inputs["w3"], dtype=np.float32)
    w1s = np.asarray(inputs["w1s"], dtype=np.float32)
    w2s = np.asarray(inputs["w2s"], dtype=np.float32)
    w3s = np.asarray(inputs["w3s"], dtype=np.float32)

    B, T, _ = x.shape
    N = B * T
    assert N == NTOK, f"kernel compiled for {NTOK} tokens, got {N}"
    flat = x.reshape(N, D)

    # ---- gate (host, f32, mirrors reference semantics) ----
    logits = flat @ gate_w                              # [N, E]
    scores = np.sqrt(np.logaddexp(np.float32(0.0), logits)).astype(np.float32)
    routed = scores + gate_bias
    idx = np.argsort(-routed, axis=1, kind="stable")[:, :TOPK]      # [N, K]
    wts = np.take_along_axis(scores, idx, axis=1)
    wts = wts / np.clip(wts.sum(axis=1, keepdims=True), 1e-6, None)

    # ---- dispatch: per-expert token lists ----
    ee = idx.reshape(-1)
    tok = np.repeat(np.arange(N), TOPK)
    ww = wts.reshape(-1).astype(np.float32)
    toks, cwts, counts = [], [], []
    for e in range(E):
        sel = ee == e
        toks.append(tok[sel])
        cwts.append(ww[sel])
        counts.append(int(sel.sum()))
    C = max(128, ((max(counts) + 63) // 64) * 64)

    # ---- per-core input maps ----
    xt_h = _ptile(flat.T.astype(BF16))                  # [128, 6, N]
    in_maps = []
    for e in range(E):
        ce = counts[e]
        xg_full = np.zeros((C, D), dtype=np.float32)
        xg_full[:ce] = flat[toks[e]]
        cpad = ((C + 127) // 128) * 128
        cw_full = np.zeros(cpad, dtype=np.float32)
        cw_full[:ce] = cwts[e]

        w13 = np.empty((2 * MI, 128, DTILES, 128), dtype=BF16)
        w13[0::2] = _slabify(w1[e].astype(BF16))
        w13[1::2] = _slabify(w3[e].astype(BF16))
        sl = slice(e * ISH, (e + 1) * ISH)
        w13s = np.empty((2 * MS, 128, DTILES, 128), dtype=BF16)
        w13s[0::2] = _slabify(w1s[:, sl].astype(BF16))
        w13s[1::2] = _slabify(w3s[:, sl].astype(BF16))

        in_maps.append({
            "w13": w13,
            "w2": _ptile(w2[e].astype(BF16)),           # [128, 24, 768]
            "w13s": w13s,
            "w2s": _ptile(w2s[sl].astype(BF16)),        # [128, 3, 768]
            "xt": xt_h,
            "xg": _ptile(xg_full.T.astype(BF16)),       # [128, 6, C]
            "cw": np.ascontiguousarray(
                cw_full.reshape(-1, 128).T),            # [128, ceil(C/128)]
        })

    # ---- build + run ----
    if C not in _BUILD_CACHE:
        _BUILD_CACHE[C] = _build(C)
    nc = _BUILD_CACHE[C]
    last_results = run_bass_kernel_spmd(nc, in_maps, core_ids=list(range(NCORES)))
    res = last_results.results

    # ---- combine (host): sum shared partials, scatter routed outputs ----
    out = res[0]["out_s"].astype(np.float32).copy()
    for c in range(1, NCORES):
        out += res[c]["out_s"]
    for e in range(E):
        ce = counts[e]
        if ce:
            out[toks[e]] += res[e]["out_r"][:ce]
    return out.reshape(B, T, D).astype(np.float32)


# revision 15
# speedup vs baseline: 1.1495x; 1.0099x over previous
"""DeepSeekMoE forward on 8 TRN2 NeuronCores.

Strategy (expert-parallel, per the sharding hint):
  - Host computes the (tiny) gate: scores = sqrt(softplus(x @ gate_w)),
    top-2 selection, normalized combine weights, and builds per-expert
    token lists (the "all-to-all dispatch" done host-side since kernel()
    receives full inputs and returns the full output).
  - Core e holds routed expert e's weights and processes the tokens
    routed to it (padded to a common capacity C).
  - The shared expert is split along its intermediate dim I across the
    8 cores (each core computes a 384-wide slice for ALL tokens); the
    partial outputs sum to the exact shared-expert output.
  - Host scatters/sums the per-core outputs back to [B, T, D].

Device compute is bf16 (f32 PSUM accumulation): TRN2 PE does bf16 at
1 cycle/row vs 4 for fp32, and bf16 halves the HBM traffic.
"""

import math

import numpy as np
import ml_dtypes

import concourse.bass as bass
import concourse.tile as tile
from concourse import bacc, mybir
from concourse.bass_utils import run_bass_kernel_spmd

BF16 = np.dtype(ml_dtypes.bfloat16)
DT_BF16 = mybir.dt.bfloat16
DT_F32 = mybir.dt.float32

D = 768            # n_embd
I = 3072           # moe_intermediate_size
E = 8              # n_routed_experts
TOPK = 2
LIMIT = 10.0
NTOK = 2048        # B*T
NCORES = 8
ISH = I // NCORES  # shared-expert I slice per core (384)
DTILES = D // 128  # 6
MI = I // 128      # 24 routed i-tiles
MS = ISH // 128    # 3 shared i-tiles

_BUILD_CACHE: dict = {}
last_results = None  # BassKernelResults of the most recent run (for test.py)


def _chunks(total, step=512):
    out = []
    t0 = 0
    while t0 < total:
        out.append((t0, min(step, total - t0)))
        t0 += step
    return out


def _build(C):
    """Build the SPMD Bass graph for capacity C (tokens per routed expert)."""
    nc = bacc.Bacc("TRN2", target_bir_lowering=False, debug=False)

    ap = lambda name, shape, dt, kind: nc.dram_tensor(name, shape, dt, kind=kind).ap()
    w13 = ap("w13", [2 * MI, 128, DTILES, 128], DT_BF16, "ExternalInput")
    w2 = ap("w2", [128, MI, D], DT_BF16, "ExternalInput")
    w13s = ap("w13s", [2 * MS, 128, DTILES, 128], DT_BF16, "ExternalInput")
    w2s = ap("w2s", [128, MS, D], DT_BF16, "ExternalInput")
    xt = ap("xt", [128, DTILES, NTOK], DT_BF16, "ExternalInput")
    xg = ap("xg", [128, DTILES, C], DT_BF16, "ExternalInput")
    cw = ap("cw", [128, (C + 127) // 128], DT_F32, "ExternalInput")
    out_r = ap("out_r", [D, C], DT_F32, "ExternalOutput")
    out_s = ap("out_s", [NTOK, D], DT_F32, "ExternalOutput")

    TCR = _chunks(C)      # routed token chunks
    TCS = _chunks(NTOK)   # shared token chunks
    DC = _chunks(D)       # output d chunks (512, 256)

    MIN = mybir.AluOpType.min
    MAX = mybir.AluOpType.max
    SILU = mybir.ActivationFunctionType.Silu
    COPY = mybir.ActivationFunctionType.Copy

    with tile.TileContext(nc) as tc:
        with (
            tc.tile_pool(name="res", bufs=1) as res,
            tc.tile_pool(name="slab", bufs=6) as slabs,
            tc.tile_pool(name="tmp", bufs=4) as tmps,
            tc.tile_pool(name="ev", bufs=4) as evs,
            tc.tile_pool(name="ps", bufs=8, space="PSUM") as ps1,
        ):
            ps2 = ps1
            # xg first: it gates the very first matmul. The other resident
            # tensors are needed only by later phases — their DMAs are
            # issued mid-way through the GEMM1 loop (side_loads) so the
            # startup slab stream gets the full HBM bandwidth.
            xg_sb = res.tile([128, DTILES, C], DT_BF16)
            for d in range(DTILES):
                nc.sync.dma_start(xg_sb[:, d, :], xg[:, d, :])

            # PE warm-up: the HAM clock gate needs ~3.4us of sustained
            # activity to lift the PE from 1.2 to 2.4 GHz. Run dummy
            # matmuls on a zeroed tile while the first DMAs land so the
            # real matmuls start warm.
            warm = res.tile([128, 512], DT_BF16)
            nc.vector.memset(warm[:], 0.0)
            pw = ps1.tile([128, 512], DT_F32, tag="ps", name="pw")
            for i in range(16):
                nc.tensor.matmul(pw[:], warm[:, :128], warm[:],
                                 start=(i == 0), stop=(i == 15))
            xt_sb = res.tile([128, DTILES, NTOK], DT_BF16)
            w2_sb = res.tile([128, MI, D], DT_BF16)
            w2s_sb = res.tile([128, MS, D], DT_BF16)
            h_sb = res.tile([128, MI, C], DT_BF16)
            hs_sb = res.tile([128, MS, NTOK], DT_BF16)

            def gemm1(npairs, wsrc, x_sb, tchunks, hout, side_loads={}):
                # hout[i, t] = silu(min(W1.T x, L)) * clip(W3.T x, -L, L)
                for m in range(npairs):
                    for fn in side_loads.get(m, []):
                        fn()
                    sg = slabs.tile([128, DTILES, 128], DT_BF16, tag="slab")
                    nc.sync.dma_start(sg[:], wsrc[2 * m])
                    su = slabs.tile([128, DTILES, 128], DT_BF16, tag="slab")
                    nc.sync.dma_start(su[:], wsrc[2 * m + 1])
                    for (t0, tl) in tchunks:
                        pg = ps1.tile([128, 512], DT_F32, tag="ps", name="pg")[:, :tl]
                        pu = ps1.tile([128, 512], DT_F32, tag="ps", name="pu")[:, :tl]
                        for d in range(DTILES):
                            nc.tensor.matmul(
                                pg[:], sg[:, d, :], x_sb[:, d, t0:t0 + tl],
                                start=(d == 0), stop=(d == DTILES - 1))
                        for d in range(DTILES):
                            nc.tensor.matmul(
                                pu[:], su[:, d, :], x_sb[:, d, t0:t0 + tl],
                                start=(d == 0), stop=(d == DTILES - 1))
                        tg = tmps.tile([128, 512], DT_F32, tag="tg", name="tg")[:, :tl]
                        nc.vector.tensor_scalar(tg[:], pg[:], LIMIT, None, MIN)
                        sa = tmps.tile([128, 512], DT_F32, tag="sa", name="sa")[:, :tl]
                        nc.scalar.activation(sa[:], tg[:], SILU)
                        tu = tmps.tile([128, 512], DT_F32, tag="tu", name="tu")[:, :tl]
                        nc.vector.tensor_scalar(tu[:], pu[:], LIMIT, -LIMIT, MIN, MAX)
                        nc.vector.tensor_mul(hout[:, m, t0:t0 + tl], sa[:], tu[:])

            def gemm2T(nitiles, h, w2sb, tlen_total, dst):
                # dst[d, t] = w2.T @ h — transposed output layout; PE cost
                # scales with tlen_total itself, not its 128-padded tiles.
                # The combine-weight scaling happens on the host instead.
                for (t0, tl) in _chunks(tlen_total):
                    for dt_ in range(DTILES):
                        ps = ps2.tile([128, 512], DT_F32, tag="ps", name="pt")[:, :tl]
                        for m in range(nitiles):
                            nc.tensor.matmul(
                                ps[:], w2sb[:, m, dt_ * 128:(dt_ + 1) * 128],
                                h[:, m, t0:t0 + tl],
                                start=(m == 0), stop=(m == nitiles - 1))
                        ev = evs.tile([128, 512], DT_F32, tag="ev", name="ev")[:, :tl]
                        if dt_ % 2 == 0:
                            nc.vector.tensor_copy(ev[:], ps[:])
                        else:
                            nc.scalar.activation(ev[:], ps[:], COPY)
                        nc.sync.dma_start(dst[dt_ * 128:(dt_ + 1) * 128, t0:t0 + tl], ev[:])

            def gemm2(nitiles, h, w2sb, tlen_total, scale_col, dst, alt_evict=False):
                # dst[t, d] = (h.T @ w2) * cw[t]
                for tt, (t0, tl) in enumerate(_chunks(tlen_total, 128)):
                    for di, (d0, dl) in enumerate(DC):
                        ps = ps2.tile([128, 512], DT_F32, tag="ps", name="po")[:tl, :dl]
                        for m in range(nitiles):
                            nc.tensor.matmul(
                                ps[:], h[:, m, t0:t0 + tl],
                                w2sb[:, m, d0:d0 + dl],
                                start=(m == 0), stop=(m == nitiles - 1))
                        ev = evs.tile([128, 512], DT_F32, tag="ev", name="ev")[:tl, :dl]
                        if scale_col is not None:
                            nc.scalar.activation(ev[:], ps[:], COPY,
                                                 scale=scale_col[:tl, tt:tt + 1])
                        elif alt_evict and di % 2 == 0:
                            nc.vector.tensor_copy(ev[:], ps[:])
                        else:
                            nc.scalar.activation(ev[:], ps[:], COPY)
                        nc.sync.dma_start(dst[t0:t0 + tl, d0:d0 + dl], ev[:])

            # Interleave the later-phase resident loads into the slab DMA
            # FIFO in small chunks so they never starve the slab stream.
            side = {}
            for j, d in enumerate(range(DTILES)):
                side.setdefault(4 + 2 * j, []).append(
                    lambda d=d: nc.sync.dma_start(xt_sb[:, d, :], xt[:, d, :]))
            for j in range(4):
                side.setdefault(16 + 2 * j, []).append(
                    lambda j=j: nc.sync.dma_start(
                        w2_sb[:, 6 * j:6 * (j + 1), :], w2[:, 6 * j:6 * (j + 1), :]))
            side.setdefault(13, []).append(
                lambda: nc.sync.dma_start(w2s_sb[:], w2s[:]))
            gemm1(MI, w13, xg_sb, TCR, h_sb, side)
            gemm1(MS, w13s, xt_sb, TCS, hs_sb)
            gemm2T(MI, h_sb, w2_sb, C, out_r)
            gemm2(MS, hs_sb, w2s_sb, NTOK, None, out_s, alt_evict=True)

    nc.compile()
    return nc


def _slabify(w):
    """[768, ncols] -> [ncols//128, 128, 6, 128] stationary slabs.

    slab[m, p, a, f] = w[a*128 + p, m*128 + f]
    """
    ncols = w.shape[1]
    return np.ascontiguousarray(
        w.reshape(DTILES, 128, ncols // 128, 128).transpose(2, 1, 0, 3))


def _ptile(a):
    """[R, cols] with R = n*128 -> [128, n, cols] (partition-major)."""
    r, c = a.shape
    return np.ascontiguousarray(a.reshape(r // 128, 128, c).transpose(1, 0, 2))


def kernel(**inputs) -> np.ndarray:
    global last_results
    x = np.asarray(inputs["x"], dtype=np.float32)
    gate_w = np.asarray(inputs["gate_w"], dtype=np.float32)
    gate_bias = np.asarray(inputs["gate_bias"], dtype=np.float32)
    w1 = np.asarray(inputs["w1"], dtype=np.float32)
    w2 = np.asarray(inputs["w2"], dtype=np.float32)
    w3 = np.asarray(inputs["w3"], dtype=np.float32)
    w1s = np.asarray(inputs["w1s"], dtype=np.float32)
    w2s = np.asarray(inputs["w2s"], dtype=np.float32)
    w3s = np.asarray(inputs["w3s"], dtype=np.float32)

    B, T, _ = x.shape
    N = B * T
    assert N == NTOK, f"kernel compiled for {NTOK} tokens, got {N}"
    flat = x.reshape(N, D)

    # ---- gate (host, f32, mirrors reference semantics) ----
    logits = flat @ gate_w                              # [N, E]
    scores = np.sqrt(np.logaddexp(np.float32(0.0), logits)).astype(np.float32)
    routed = scores + gate_bias
    idx = np.argsort(-routed, axis=1, kind="stable")[:, :TOPK]      # [N, K]
    wts = np.take_along_axis(scores, idx, axis=1)
    wts = wts / np.clip(wts.sum(axis=1, keepdims=True), 1e-6, None)

    # ---- dispatch: per-expert token lists ----
    ee = idx.reshape(-1)
    tok = np.repeat(np.arange(N), TOPK)
    ww = wts.reshape(-1).astype(np.float32)
    toks, cwts, counts = [], [], []
    for e in range(E):
        sel = ee == e
        toks.append(tok[sel])
        cwts.append(ww[sel])
        counts.append(int(sel.sum()))
    C = max(128, ((max(counts) + 63) // 64) * 64)

    # ---- per-core input maps ----
    xt_h = _ptile(flat.T.astype(BF16))                  # [128, 6, N]
    in_maps = []
    for e in range(E):
        ce = counts[e]
        xg_full = np.zeros((C, D), dtype=np.float32)
        xg_full[:ce] = flat[toks[e]]
        cpad = ((C + 127) // 128) * 128
        cw_full = np.zeros(cpad, dtype=np.float32)
        cw_full[:ce] = cwts[e]

        w13 = np.empty((2 * MI, 128, DTILES, 128), dtype=BF16)
        w13[0::2] = _slabify(w1[e].astype(BF16))
        w13[1::2] = _slabify(w3[e].astype(BF16))
        sl = slice(e * ISH, (e + 1) * ISH)
        w13s = np.empty((2 * MS, 128, DTILES, 128), dtype=BF16)
        w13s[0::2] = _slabify(w1s[:, sl].astype(BF16))
        w13s[1::2] = _slabify(w3s[:, sl].astype(BF16))

        in_maps.append({
            "w13": w13,
            "w2": _ptile(w2[e].astype(BF16)),           # [128, 24, 768]
            "w13s": w13s,
            "w2s": _ptile(w2s[sl].astype(BF16)),        # [128, 3, 768]
            "xt": xt_h,
            "xg": _ptile(xg_full.T.astype(BF16)),       # [128, 6, C]
            "cw": np.ascontiguousarray(
                cw_full.reshape(-1, 128).T),            # [128, ceil(C/128)]
        })

    # ---- build + run ----
    if C not in _BUILD_CACHE:
        _BUILD_CACHE[C] = _build(C)
    nc = _BUILD_CACHE[C]
    last_results = run_bass_kernel_spmd(nc, in_maps, core_ids=list(range(NCORES)))
    res = last_results.results

    # ---- combine (host): sum shared partials, scatter routed outputs ----
    out = res[0]["out_s"].astype(np.float32).copy()
    for c in range(1, NCORES):
        out += res[c]["out_s"]
    for e in range(E):
        ce = counts[e]
        if ce:
            out[toks[e]] += res[e]["out_r"][:, :ce].T * cwts[e][:, None]
    return out.reshape(B, T, D).astype(np.float32)


# revision 18
# speedup vs baseline: 1.2163x; 1.0582x over previous
"""DeepSeekMoE forward on 8 TRN2 NeuronCores.

Strategy (expert-parallel, per the sharding hint):
  - Host computes the (tiny) gate: scores = sqrt(softplus(x @ gate_w)),
    top-2 selection, normalized combine weights, and builds per-expert
    token lists (the "all-to-all dispatch" done host-side since kernel()
    receives full inputs and returns the full output).
  - Core e holds routed expert e's weights and processes the tokens
    routed to it (padded to a common capacity C).
  - The shared expert is split along its intermediate dim I across the
    8 cores (each core computes a 384-wide slice for ALL tokens); the
    partial outputs sum to the exact shared-expert output.
  - Host scatters/sums the per-core outputs back to [B, T, D].

Device compute is bf16 (f32 PSUM accumulation): TRN2 PE does bf16 at
1 cycle/row vs 4 for fp32, and bf16 halves the HBM traffic.
"""

import math

import numpy as np
import ml_dtypes

import concourse.bass as bass
import concourse.tile as tile
from concourse import bacc, mybir
from concourse.bass_utils import run_bass_kernel_spmd

BF16 = np.dtype(ml_dtypes.bfloat16)
DT_BF16 = mybir.dt.bfloat16
DT_F32 = mybir.dt.float32

D = 768            # n_embd
I = 3072           # moe_intermediate_size
E = 8              # n_routed_experts
TOPK = 2
LIMIT = 10.0
NTOK = 2048        # B*T
NCORES = 8
ISH = I // NCORES  # shared-expert I slice per core (384)
DTILES = D // 128  # 6
MI = I // 128      # 24 routed i-tiles
MS = ISH // 128    # 3 shared i-tiles

_BUILD_CACHE: dict = {}
last_results = None  # BassKernelResults of the most recent run (for test.py)


def _chunks(total, step=512):
    out = []
    t0 = 0
    while t0 < total:
        out.append((t0, min(step, total - t0)))
        t0 += step
    return out


def _build(C):
    """Build the SPMD Bass graph for capacity C (tokens per routed expert)."""
    nc = bacc.Bacc("TRN2", target_bir_lowering=False, debug=False)

    ap = lambda name, shape, dt, kind: nc.dram_tensor(name, shape, dt, kind=kind).ap()
    w13 = ap("w13", [2 * MI, 128, DTILES, 128], DT_BF16, "ExternalInput")
    w2 = ap("w2", [128, MI, D], DT_BF16, "ExternalInput")
    w13s = ap("w13s", [2 * MS, 128, DTILES, 128], DT_BF16, "ExternalInput")
    w2s = ap("w2s", [128, MS, D], DT_BF16, "ExternalInput")
    xt = ap("xt", [128, DTILES, NTOK], DT_BF16, "ExternalInput")
    xg = ap("xg", [128, DTILES, C], DT_BF16, "ExternalInput")
    cw = ap("cw", [128, (C + 127) // 128], DT_F32, "ExternalInput")
    out_r = ap("out_r", [D, C], DT_F32, "ExternalOutput")
    out_s = ap("out_s", [NTOK, D], DT_F32, "ExternalOutput")

    TCR = _chunks(C)      # routed token chunks
    TCS = _chunks(NTOK)   # shared token chunks
    DC = _chunks(D)       # output d chunks (512, 256)

    MIN = mybir.AluOpType.min
    MAX = mybir.AluOpType.max
    SILU = mybir.ActivationFunctionType.Silu
    COPY = mybir.ActivationFunctionType.Copy

    with tile.TileContext(nc) as tc:
        with (
            tc.tile_pool(name="res", bufs=1) as res,
            tc.tile_pool(name="slab", bufs=6) as slabs,
            tc.tile_pool(name="tmp", bufs=4) as tmps,
            tc.tile_pool(name="ev", bufs=4) as evs,
            tc.tile_pool(name="ps", bufs=8, space="PSUM") as ps1,
        ):
            ps2 = ps1
            # xg first: it gates the very first matmul. The other resident
            # tensors are needed only by later phases — their DMAs are
            # issued mid-way through the GEMM1 loop (side_loads) so the
            # startup slab stream gets the full HBM bandwidth.
            xg_sb = res.tile([128, DTILES, C], DT_BF16)
            for d in range(DTILES):
                nc.sync.dma_start(xg_sb[:, d, :], xg[:, d, :])

            # PE warm-up: the HAM clock gate needs ~3.4us of sustained
            # activity to lift the PE from 1.2 to 2.4 GHz. Run dummy
            # matmuls on a zeroed tile while the first DMAs land so the
            # real matmuls start warm.
            warm = res.tile([128, 512], DT_BF16)
            nc.vector.memset(warm[:], 0.0)
            pw = ps1.tile([128, 512], DT_F32, tag="ps", name="pw")
            for i in range(16):
                nc.tensor.matmul(pw[:], warm[:, :128], warm[:],
                                 start=(i == 0), stop=(i == 15))
            xt_sb = res.tile([128, DTILES, NTOK], DT_BF16)
            w2_sb = res.tile([128, MI, D], DT_BF16)
            w2s_sb = res.tile([128, MS, D], DT_BF16)
            h_sb = res.tile([128, MI, C], DT_BF16)
            hs_sb = res.tile([128, MS, NTOK], DT_BF16)

            def gemm1(npairs, wsrc, x_sb, tchunks, hout, side_loads={}):
                # hout[i, t] = silu(min(W1.T x, L)) * clip(W3.T x, -L, L)
                for m in range(npairs):
                    for fn in side_loads.get(m, []):
                        fn()
                    sg = slabs.tile([128, DTILES, 128], DT_BF16, tag="slab")
                    nc.sync.dma_start(sg[:], wsrc[2 * m])
                    su = slabs.tile([128, DTILES, 128], DT_BF16, tag="slab")
                    nc.sync.dma_start(su[:], wsrc[2 * m + 1])
                    for (t0, tl) in tchunks:
                        pg = ps1.tile([128, 512], DT_F32, tag="ps", name="pg")[:, :tl]
                        pu = ps1.tile([128, 512], DT_F32, tag="ps", name="pu")[:, :tl]
                        for d in range(DTILES):
                            nc.tensor.matmul(
                                pg[:], sg[:, d, :], x_sb[:, d, t0:t0 + tl],
                                start=(d == 0), stop=(d == DTILES - 1))
                        for d in range(DTILES):
                            nc.tensor.matmul(
                                pu[:], su[:, d, :], x_sb[:, d, t0:t0 + tl],
                                start=(d == 0), stop=(d == DTILES - 1))
                        tg = tmps.tile([128, 512], DT_F32, tag="tg", name="tg")[:, :tl]
                        nc.vector.tensor_scalar(tg[:], pg[:], LIMIT, None, MIN)
                        sa = tmps.tile([128, 512], DT_F32, tag="sa", name="sa")[:, :tl]
                        nc.scalar.activation(sa[:], tg[:], SILU)
                        tu = tmps.tile([128, 512], DT_F32, tag="tu", name="tu")[:, :tl]
                        nc.vector.tensor_scalar(tu[:], pu[:], LIMIT, -LIMIT, MIN, MAX)
                        nc.vector.tensor_mul(hout[:, m, t0:t0 + tl], sa[:], tu[:])

            def gemm2T_units(nitiles, h, w2sb, tlen_total, dst):
                # dst[d, t] = w2.T @ h — transposed output layout; PE cost
                # scales with tlen_total itself, not its 128-padded tiles.
                # The combine-weight scaling happens on the host instead.
                for (t0, tl) in _chunks(tlen_total):
                    for dt_ in range(DTILES):
                        def unit(t0=t0, tl=tl, dt_=dt_):
                            ps = ps2.tile([128, 512], DT_F32, tag="ps", name="pt")[:, :tl]
                            for m in range(nitiles):
                                nc.tensor.matmul(
                                    ps[:], w2sb[:, m, dt_ * 128:(dt_ + 1) * 128],
                                    h[:, m, t0:t0 + tl],
                                    start=(m == 0), stop=(m == nitiles - 1))
                            ev = evs.tile([128, 512], DT_F32, tag="ev", name="ev")[:, :tl]
                            if dt_ % 2 == 0:
                                nc.vector.tensor_copy(ev[:], ps[:])
                            else:
                                nc.scalar.activation(ev[:], ps[:], COPY)
                            nc.sync.dma_start(
                                dst[dt_ * 128:(dt_ + 1) * 128, t0:t0 + tl], ev[:])
                        yield unit

            def gemm2_units(nitiles, h, w2sb, tlen_total, dst):
                # dst[t, d] = h.T @ w2
                for tt, (t0, tl) in enumerate(_chunks(tlen_total, 128)):
                    for di, (d0, dl) in enumerate(DC):
                        def unit(t0=t0, tl=tl, di=di, d0=d0, dl=dl):
                            ps = ps2.tile([128, 512], DT_F32, tag="ps", name="po")[:tl, :dl]
                            for m in range(nitiles):
                                nc.tensor.matmul(
                                    ps[:], h[:, m, t0:t0 + tl],
                                    w2sb[:, m, d0:d0 + dl],
                                    start=(m == 0), stop=(m == nitiles - 1))
                            ev = evs.tile([128, 512], DT_F32, tag="ev", name="ev")[:tl, :dl]
                            if di % 2 == 0:
                                nc.vector.tensor_copy(ev[:], ps[:])
                            else:
                                nc.scalar.activation(ev[:], ps[:], COPY)
                            nc.sync.dma_start(dst[t0:t0 + tl, d0:d0 + dl], ev[:])
                        yield unit

            # Interleave the later-phase resident loads into the slab DMA
            # FIFO in small chunks so they never starve the slab stream.
            side = {}
            for j, d in enumerate(range(DTILES)):
                side.setdefault(4 + 2 * j, []).append(
                    lambda d=d: nc.sync.dma_start(xt_sb[:, d, :], xt[:, d, :]))
            for j in range(4):
                side.setdefault(16 + 2 * j, []).append(
                    lambda j=j: nc.sync.dma_start(
                        w2_sb[:, 6 * j:6 * (j + 1), :], w2[:, 6 * j:6 * (j + 1), :]))
            side.setdefault(13, []).append(
                lambda: nc.sync.dma_start(w2s_sb[:], w2s[:]))
            gemm1(MI, w13, xg_sb, TCR, h_sb, side)
            gemm1(MS, w13s, xt_sb, TCS, hs_sb)
            # Interleave the evict-heavy shared GEMM2 (many small psum
            # groups) with the evict-light routed GEMM2 (long psum
            # accumulations) so the eviction pipeline drains while PE is
            # still busy, and the kernel ends on an evict-light unit.
            r_units = list(gemm2T_units(MI, h_sb, w2_sb, C, out_r))
            s_units = list(gemm2_units(MS, hs_sb, w2s_sb, NTOK, out_s))
            ns, nr = len(s_units), len(r_units)
            si = 0
            for ri, ru in enumerate(r_units):
                take = (ns * (ri + 1)) // nr
                while si < min(take, ns):
                    s_units[si]()
                    si += 1
                ru()
            while si < ns:
                s_units[si]()
                si += 1

    nc.compile()
    return nc


def _slabify(w):
    """[768, ncols] -> [ncols//128, 128, 6, 128] stationary slabs.

    slab[m, p, a, f] = w[a*128 + p, m*128 + f]
    """
    ncols = w.shape[1]
    return np.ascontiguousarray(
        w.reshape(DTILES, 128, ncols // 128, 128).transpose(2, 1, 0, 3))


def _ptile(a):
    """[R, cols] with R = n*128 -> [128, n, cols] (partition-major)."""
    r, c = a.shape
    return np.ascontiguousarray(a.reshape(r // 128, 128, c).transpose(1, 0, 2))


def kernel(**inputs) -> np.ndarray:
    global last_results
    x = np.asarray(inputs["x"], dtype=np.float32)
    gate_w = np.asarray(inputs["gate_w"], dtype=np.float32)
    gate_bias = np.asarray(inputs["gate_bias"], dtype=np.float32)
    w1 = np.asarray(inputs["w1"], dtype=np.float32)
    w2 = np.asarray(inputs["w2"], dtype=np.float32)
    w3 = np.asarray(inputs["w3"], dtype=np.float32)
    w1s = np.asarray(inputs["w1s"], dtype=np.float32)
    w2s = np.asarray(inputs["w2s"], dtype=np.float32)
    w3s = np.asarray(inputs["w3s"], dtype=np.float32)

    B, T, _ = x.shape
    N = B * T
    assert N == NTOK, f"kernel compiled for {NTOK} tokens, got {N}"
    flat = x.reshape(N, D)

    # ---- gate (host, f32, mirrors reference semantics) ----
    logits = flat @ gate_w                              # [N, E]
    scores = np.sqrt(np.logaddexp(np.float32(0.0), logits)).astype(np.float32)
    routed = scores + gate_bias
    idx = np.argsort(-routed, axis=1, kind="stable")[:, :TOPK]      # [N, K]
    wts = np.take_along_axis(scores, idx, axis=1)
    wts = wts / np.clip(wts.sum(axis=1, keepdims=True), 1e-6, None)

    # ---- dispatch: per-expert token lists ----
    ee = idx.reshape(-1)
    tok = np.repeat(np.arange(N), TOPK)
    ww = wts.reshape(-1).astype(np.float32)
    toks, cwts, counts = [], [], []
    for e in range(E):
        sel = ee == e
        toks.append(tok[sel])
        cwts.append(ww[sel])
        counts.append(int(sel.sum()))
    C = max(128, ((max(counts) + 63) // 64) * 64)

    # ---- per-core input maps ----
    xt_h = _ptile(flat.T.astype(BF16))                  # [128, 6, N]
    in_maps = []
    for e in range(E):
        ce = counts[e]
        xg_full = np.zeros((C, D), dtype=np.float32)
        xg_full[:ce] = flat[toks[e]]
        cpad = ((C + 127) // 128) * 128
        cw_full = np.zeros(cpad, dtype=np.float32)
        cw_full[:ce] = cwts[e]

        w13 = np.empty((2 * MI, 128, DTILES, 128), dtype=BF16)
        w13[0::2] = _slabify(w1[e].astype(BF16))
        w13[1::2] = _slabify(w3[e].astype(BF16))
        sl = slice(e * ISH, (e + 1) * ISH)
        w13s = np.empty((2 * MS, 128, DTILES, 128), dtype=BF16)
        w13s[0::2] = _slabify(w1s[:, sl].astype(BF16))
        w13s[1::2] = _slabify(w3s[:, sl].astype(BF16))

        in_maps.append({
            "w13": w13,
            "w2": _ptile(w2[e].astype(BF16)),           # [128, 24, 768]
            "w13s": w13s,
            "w2s": _ptile(w2s[sl].astype(BF16)),        # [128, 3, 768]
            "xt": xt_h,
            "xg": _ptile(xg_full.T.astype(BF16)),       # [128, 6, C]
            "cw": np.ascontiguousarray(
                cw_full.reshape(-1, 128).T),            # [128, ceil(C/128)]
        })

    # ---- build + run ----
    if C not in _BUILD_CACHE:
        _BUILD_CACHE[C] = _build(C)
    nc = _BUILD_CACHE[C]
    last_results = run_bass_kernel_spmd(nc, in_maps, core_ids=list(range(NCORES)))
    res = last_results.results

    # ---- combine (host): sum shared partials, scatter routed outputs ----
    out = res[0]["out_s"].astype(np.float32).copy()
    for c in range(1, NCORES):
        out += res[c]["out_s"]
    for e in range(E):
        ce = counts[e]
        if ce:
            out[toks[e]] += res[e]["out_r"][:, :ce].T * cwts[e][:, None]
    return out.reshape(B, T, D).astype(np.float32)


# revision 19
# speedup vs baseline: 1.2179x; 1.0013x over previous
"""DeepSeekMoE forward on 8 TRN2 NeuronCores.

Strategy (expert-parallel, per the sharding hint):
  - Host computes the (tiny) gate: scores = sqrt(softplus(x @ gate_w)),
    top-2 selection, normalized combine weights, and builds per-expert
    token lists (the "all-to-all dispatch" done host-side since kernel()
    receives full inputs and returns the full output).
  - Core e holds routed expert e's weights and processes the tokens
    routed to it (padded to a common capacity C).
  - The shared expert is split along its intermediate dim I across the
    8 cores (each core computes a 384-wide slice for ALL tokens); the
    partial outputs sum to the exact shared-expert output.
  - Host scatters/sums the per-core outputs back to [B, T, D].

Device compute is bf16 (f32 PSUM accumulation): TRN2 PE does bf16 at
1 cycle/row vs 4 for fp32, and bf16 halves the HBM traffic.
"""

import math

import numpy as np
import ml_dtypes

import concourse.bass as bass
import concourse.tile as tile
from concourse import bacc, mybir
from concourse.bass_utils import run_bass_kernel_spmd

BF16 = np.dtype(ml_dtypes.bfloat16)
DT_BF16 = mybir.dt.bfloat16
DT_F32 = mybir.dt.float32

D = 768            # n_embd
I = 3072           # moe_intermediate_size
E = 8              # n_routed_experts
TOPK = 2
LIMIT = 10.0
NTOK = 2048        # B*T
NCORES = 8
ISH = I // NCORES  # shared-expert I slice per core (384)
DTILES = D // 128  # 6
MI = I // 128      # 24 routed i-tiles
MS = ISH // 128    # 3 shared i-tiles

_BUILD_CACHE: dict = {}
last_results = None  # BassKernelResults of the most recent run (for test.py)


def _chunks(total, step=512):
    out = []
    t0 = 0
    while t0 < total:
        out.append((t0, min(step, total - t0)))
        t0 += step
    return out


def _build(C):
    """Build the SPMD Bass graph for capacity C (tokens per routed expert)."""
    nc = bacc.Bacc("TRN2", target_bir_lowering=False, debug=False)

    ap = lambda name, shape, dt, kind: nc.dram_tensor(name, shape, dt, kind=kind).ap()
    w13 = ap("w13", [2 * MI, 128, DTILES, 128], DT_BF16, "ExternalInput")
    w2 = ap("w2", [128, MI, D], DT_BF16, "ExternalInput")
    w13s = ap("w13s", [2 * MS, 128, DTILES, 128], DT_BF16, "ExternalInput")
    w2s = ap("w2s", [128, MS, D], DT_BF16, "ExternalInput")
    xt = ap("xt", [128, DTILES, NTOK], DT_BF16, "ExternalInput")
    xg = ap("xg", [128, DTILES, C], DT_BF16, "ExternalInput")
    cw = ap("cw", [128, (C + 127) // 128], DT_F32, "ExternalInput")
    out_r = ap("out_r", [D, C], DT_F32, "ExternalOutput")
    out_s = ap("out_s", [NTOK, D], DT_F32, "ExternalOutput")

    TCR = _chunks(C)      # routed token chunks
    TCS = _chunks(NTOK)   # shared token chunks
    DC = _chunks(D)       # output d chunks (512, 256)

    MIN = mybir.AluOpType.min
    MAX = mybir.AluOpType.max
    SILU = mybir.ActivationFunctionType.Silu
    COPY = mybir.ActivationFunctionType.Copy

    with tile.TileContext(nc) as tc:
        with (
            tc.tile_pool(name="res", bufs=1) as res,
            tc.tile_pool(name="slab", bufs=8) as slabs,
            tc.tile_pool(name="tmp", bufs=4) as tmps,
            tc.tile_pool(name="ev", bufs=4) as evs,
            tc.tile_pool(name="ps", bufs=8, space="PSUM") as ps1,
        ):
            ps2 = ps1
            # xg first: it gates the very first matmul. The other resident
            # tensors are needed only by later phases — their DMAs are
            # issued mid-way through the GEMM1 loop (side_loads) so the
            # startup slab stream gets the full HBM bandwidth.
            xg_sb = res.tile([128, DTILES, C], DT_BF16)
            for d in range(DTILES):
                nc.sync.dma_start(xg_sb[:, d, :], xg[:, d, :])

            # PE warm-up: the HAM clock gate needs ~3.4us of sustained
            # activity to lift the PE from 1.2 to 2.4 GHz. Run dummy
            # matmuls on a zeroed tile while the first DMAs land so the
            # real matmuls start warm.
            warm = res.tile([128, 512], DT_BF16)
            nc.vector.memset(warm[:], 0.0)
            pw = ps1.tile([128, 512], DT_F32, tag="ps", name="pw")
            for i in range(16):
                nc.tensor.matmul(pw[:], warm[:, :128], warm[:],
                                 start=(i == 0), stop=(i == 15))
            xt_sb = res.tile([128, DTILES, NTOK], DT_BF16)
            w2_sb = res.tile([128, MI, D], DT_BF16)
            w2s_sb = res.tile([128, MS, D], DT_BF16)
            h_sb = res.tile([128, MI, C], DT_BF16)
            hs_sb = res.tile([128, MS, NTOK], DT_BF16)

            def gemm1(npairs, wsrc, x_sb, tchunks, hout, side_loads={}):
                # hout[i, t] = silu(min(W1.T x, L)) * clip(W3.T x, -L, L)
                for m in range(npairs):
                    for fn in side_loads.get(m, []):
                        fn()
                    sg = slabs.tile([128, DTILES, 128], DT_BF16, tag="slab")
                    nc.sync.dma_start(sg[:], wsrc[2 * m])
                    su = slabs.tile([128, DTILES, 128], DT_BF16, tag="slab")
                    nc.sync.dma_start(su[:], wsrc[2 * m + 1])
                    for (t0, tl) in tchunks:
                        pg = ps1.tile([128, 512], DT_F32, tag="ps", name="pg")[:, :tl]
                        pu = ps1.tile([128, 512], DT_F32, tag="ps", name="pu")[:, :tl]
                        for d in range(DTILES):
                            nc.tensor.matmul(
                                pg[:], sg[:, d, :], x_sb[:, d, t0:t0 + tl],
                                start=(d == 0), stop=(d == DTILES - 1))
                        for d in range(DTILES):
                            nc.tensor.matmul(
                                pu[:], su[:, d, :], x_sb[:, d, t0:t0 + tl],
                                start=(d == 0), stop=(d == DTILES - 1))
                        tg = tmps.tile([128, 512], DT_F32, tag="tg", name="tg")[:, :tl]
                        nc.vector.tensor_scalar(tg[:], pg[:], LIMIT, None, MIN)
                        sa = tmps.tile([128, 512], DT_F32, tag="sa", name="sa")[:, :tl]
                        nc.scalar.activation(sa[:], tg[:], SILU)
                        tu = tmps.tile([128, 512], DT_F32, tag="tu", name="tu")[:, :tl]
                        nc.vector.tensor_scalar(tu[:], pu[:], LIMIT, -LIMIT, MIN, MAX)
                        nc.vector.tensor_mul(hout[:, m, t0:t0 + tl], sa[:], tu[:])

            def gemm2T_units(nitiles, h, w2sb, tlen_total, dst):
                # dst[d, t] = w2.T @ h — transposed output layout; PE cost
                # scales with tlen_total itself, not its 128-padded tiles.
                # The combine-weight scaling happens on the host instead.
                for (t0, tl) in _chunks(tlen_total):
                    for dt_ in range(DTILES):
                        def unit(t0=t0, tl=tl, dt_=dt_):
                            ps = ps2.tile([128, 512], DT_F32, tag="ps", name="pt")[:, :tl]
                            for m in range(nitiles):
                                nc.tensor.matmul(
                                    ps[:], w2sb[:, m, dt_ * 128:(dt_ + 1) * 128],
                                    h[:, m, t0:t0 + tl],
                                    start=(m == 0), stop=(m == nitiles - 1))
                            ev = evs.tile([128, 512], DT_F32, tag="ev", name="ev")[:, :tl]
                            if dt_ % 2 == 0:
                                nc.vector.tensor_copy(ev[:], ps[:])
                            else:
                                nc.scalar.activation(ev[:], ps[:], COPY)
                            nc.sync.dma_start(
                                dst[dt_ * 128:(dt_ + 1) * 128, t0:t0 + tl], ev[:])
                        yield unit

            def gemm2_units(nitiles, h, w2sb, tlen_total, dst):
                # dst[t, d] = h.T @ w2
                for tt, (t0, tl) in enumerate(_chunks(tlen_total, 128)):
                    for di, (d0, dl) in enumerate(DC):
                        def unit(t0=t0, tl=tl, di=di, d0=d0, dl=dl):
                            ps = ps2.tile([128, 512], DT_F32, tag="ps", name="po")[:tl, :dl]
                            for m in range(nitiles):
                                nc.tensor.matmul(
                                    ps[:], h[:, m, t0:t0 + tl],
                                    w2sb[:, m, d0:d0 + dl],
                                    start=(m == 0), stop=(m == nitiles - 1))
                            ev = evs.tile([128, 512], DT_F32, tag="ev", name="ev")[:tl, :dl]
                            if di % 2 == 0:
                                nc.vector.tensor_copy(ev[:], ps[:])
                            else:
                                nc.scalar.activation(ev[:], ps[:], COPY)
                            nc.sync.dma_start(dst[t0:t0 + tl, d0:d0 + dl], ev[:])
                        yield unit

            # Interleave the later-phase resident loads into the slab DMA
            # FIFO in small chunks so they never starve the slab stream.
            side = {}
            for j, d in enumerate(range(DTILES)):
                side.setdefault(2 + 2 * j, []).append(
                    lambda d=d: nc.sync.dma_start(xt_sb[:, d, :], xt[:, d, :]))
            for j in range(8):
                side.setdefault(14 + j, []).append(
                    lambda j=j: nc.sync.dma_start(
                        w2_sb[:, 3 * j:3 * (j + 1), :], w2[:, 3 * j:3 * (j + 1), :]))
            side.setdefault(23, []).append(
                lambda: nc.sync.dma_start(w2s_sb[:], w2s[:]))
            gemm1(MI, w13, xg_sb, TCR, h_sb, side)
            gemm1(MS, w13s, xt_sb, TCS, hs_sb)
            # Interleave the evict-heavy shared GEMM2 (many small psum
            # groups) with the evict-light routed GEMM2 (long psum
            # accumulations) so the eviction pipeline drains while PE is
            # still busy, and the kernel ends on an evict-light unit.
            r_units = list(gemm2T_units(MI, h_sb, w2_sb, C, out_r))
            s_units = list(gemm2_units(MS, hs_sb, w2s_sb, NTOK, out_s))
            ns, nr = len(s_units), len(r_units)
            si = 0
            for ri, ru in enumerate(r_units):
                take = (ns * (ri + 1)) // nr
                while si < min(take, ns):
                    s_units[si]()
                    si += 1
                ru()
            while si < ns:
                s_units[si]()
                si += 1

    nc.compile()
    return nc


def _slabify(w):
    """[768, ncols] -> [ncols//128, 128, 6, 128] stationary slabs.

    slab[m, p, a, f] = w[a*128 + p, m*128 + f]
    """
    ncols = w.shape[1]
    return np.ascontiguousarray(
        w.reshape(DTILES, 128, ncols // 128, 128).transpose(2, 1, 0, 3))


def _ptile(a):
    """[R, cols] with R = n*128 -> [128, n, cols] (partition-major)."""
    r, c = a.shape
    return np.ascontiguousarray(a.reshape(r // 128, 128, c).transpose(1, 0, 2))


def kernel(**inputs) -> np.ndarray:
    global last_results
    x = np.asarray(inputs["x"], dtype=np.float32)
    gate_w = np.asarray(inputs["gate_w"], dtype=np.float32)
    gate_bias = np.asarray(inputs["gate_bias"], dtype=np.float32)
    w1 = np.asarray(inputs["w1"], dtype=np.float32)
    w2 = np.asarray(inputs["w2"], dtype=np.float32)
    w3 = np.asarray(inputs["w3"], dtype=np.float32)
    w1s = np.asarray(inputs["w1s"], dtype=np.float32)
    w2s = np.asarray(inputs["w2s"], dtype=np.float32)
    w3s = np.asarray(inputs["w3s"], dtype=np.float32)

    B, T, _ = x.shape
    N = B * T
    assert N == NTOK, f"kernel compiled for {NTOK} tokens, got {N}"
    flat = x.reshape(N, D)

    # ---- gate (host, f32, mirrors reference semantics) ----
    logits = flat @ gate_w                              # [N, E]
    scores = np.sqrt(np.logaddexp(np.float32(0.0), logits)).astype(np.float32)
    routed = scores + gate_bias
    idx = np.argsort(-routed, axis=1, kind="stable")[:, :TOPK]      # [N, K]
    wts = np.take_along_axis(scores, idx, axis=1)
    wts = wts / np.clip(wts.sum(axis=1, keepdims=True), 1e-6, None)

    # ---- dispatch: per-expert token lists ----
    ee = idx.reshape(-1)
    tok = np.repeat(np.arange(N), TOPK)
    ww = wts.reshape(-1).astype(np.float32)
    toks, cwts, counts = [], [], []
    for e in range(E):
        sel = ee == e
        toks.append(tok[sel])
        cwts.append(ww[sel])
        counts.append(int(sel.sum()))
    C = max(128, ((max(counts) + 31) // 32) * 32)

    # ---- per-core input maps ----
    xt_h = _ptile(flat.T.astype(BF16))                  # [128, 6, N]
    in_maps = []
    for e in range(E):
        ce = counts[e]
        xg_full = np.zeros((C, D), dtype=np.float32)
        xg_full[:ce] = flat[toks[e]]
        cpad = ((C + 127) // 128) * 128
        cw_full = np.zeros(cpad, dtype=np.float32)
        cw_full[:ce] = cwts[e]

        w13 = np.empty((2 * MI, 128, DTILES, 128), dtype=BF16)
        w13[0::2] = _slabify(w1[e].astype(BF16))
        w13[1::2] = _slabify(w3[e].astype(BF16))
        sl = slice(e * ISH, (e + 1) * ISH)
        w13s = np.empty((2 * MS, 128, DTILES, 128), dtype=BF16)
        w13s[0::2] = _slabify(w1s[:, sl].astype(BF16))
        w13s[1::2] = _slabify(w3s[:, sl].astype(BF16))

        in_maps.append({
            "w13": w13,
            "w2": _ptile(w2[e].astype(BF16)),           # [128, 24, 768]
            "w13s": w13s,
            "w2s": _ptile(w2s[sl].astype(BF16)),        # [128, 3, 768]
            "xt": xt_h,
            "xg": _ptile(xg_full.T.astype(BF16)),       # [128, 6, C]
            "cw": np.ascontiguousarray(
                cw_full.reshape(-1, 128).T),            # [128, ceil(C/128)]
        })

    # ---- build + run ----
    if C not in _BUILD_CACHE:
        _BUILD_CACHE[C] = _build(C)
    nc = _BUILD_CACHE[C]
    last_results = run_bass_kernel_spmd(nc, in_maps, core_ids=list(range(NCORES)))
    res = last_results.results

    # ---- combine (host): sum shared partials, scatter routed outputs ----
    out = res[0]["out_s"].astype(np.float32).copy()
    for c in range(1, NCORES):
        out += res[c]["out_s"]
    for e in range(E):
        ce = counts[e]
        if ce:
            out[toks[e]] += res[e]["out_r"][:, :ce].T * cwts[e][:, None]
    return out.reshape(B, T, D).astype(np.float32)


# revision 20
# speedup vs baseline: 1.2293x; 1.0093x over previous
"""DeepSeekMoE forward on 8 TRN2 NeuronCores.

Strategy (expert-parallel, per the sharding hint):
  - Host computes the (tiny) gate: scores = sqrt(softplus(x @ gate_w)),
    top-2 selection, normalized combine weights, and builds per-expert
    token lists (the "all-to-all dispatch" done host-side since kernel()
    receives full inputs and returns the full output).
  - Core e holds routed expert e's weights and processes the tokens
    routed to it (padded to a common capacity C).
  - The shared expert is split along its intermediate dim I across the
    8 cores (each core computes a 384-wide slice for ALL tokens); the
    partial outputs sum to the exact shared-expert output.
  - Host scatters/sums the per-core outputs back to [B, T, D].

Device compute is bf16 (f32 PSUM accumulation): TRN2 PE does bf16 at
1 cycle/row vs 4 for fp32, and bf16 halves the HBM traffic.
"""

import math

import numpy as np
import ml_dtypes

import concourse.bass as bass
import concourse.tile as tile
from concourse import bacc, mybir
from concourse.bass_utils import run_bass_kernel_spmd

BF16 = np.dtype(ml_dtypes.bfloat16)
DT_BF16 = mybir.dt.bfloat16
DT_F32 = mybir.dt.float32

D = 768            # n_embd
I = 3072           # moe_intermediate_size
E = 8              # n_routed_experts
TOPK = 2
LIMIT = 10.0
NTOK = 2048        # B*T
NCORES = 8
ISH = I // NCORES  # shared-expert I slice per core (384)
DTILES = D // 128  # 6
MI = I // 128      # 24 routed i-tiles
MS = ISH // 128    # 3 shared i-tiles

_BUILD_CACHE: dict = {}
last_results = None  # BassKernelResults of the most recent run (for test.py)


def _chunks(total, step=512):
    out = []
    t0 = 0
    while t0 < total:
        out.append((t0, min(step, total - t0)))
        t0 += step
    return out


def _build(C):
    """Build the SPMD Bass graph for capacity C (tokens per routed expert)."""
    nc = bacc.Bacc("TRN2", target_bir_lowering=False, debug=False)

    ap = lambda name, shape, dt, kind: nc.dram_tensor(name, shape, dt, kind=kind).ap()
    w13 = ap("w13", [2 * MI, 128, DTILES, 128], DT_BF16, "ExternalInput")
    w2 = ap("w2", [128, MI, D], DT_BF16, "ExternalInput")
    w13s = ap("w13s", [2 * MS, 128, DTILES, 128], DT_BF16, "ExternalInput")
    w2s = ap("w2s", [128, MS, D], DT_BF16, "ExternalInput")
    xt = ap("xt", [128, DTILES, NTOK], DT_BF16, "ExternalInput")
    xg = ap("xg", [128, DTILES, C], DT_BF16, "ExternalInput")
    cw = ap("cw", [128, (C + 127) // 128], DT_F32, "ExternalInput")
    out_r = ap("out_r", [D, C], DT_F32, "ExternalOutput")
    out_s = ap("out_s", [NTOK, D], DT_F32, "ExternalOutput")

    TCR = _chunks(C)      # routed token chunks
    TCS = _chunks(NTOK)   # shared token chunks
    DC = _chunks(D)       # output d chunks (512, 256)

    MIN = mybir.AluOpType.min
    MAX = mybir.AluOpType.max
    SILU = mybir.ActivationFunctionType.Silu
    COPY = mybir.ActivationFunctionType.Copy

    with tile.TileContext(nc) as tc:
        with (
            tc.tile_pool(name="res", bufs=1) as res,
            tc.tile_pool(name="slab", bufs=8) as slabs,
            tc.tile_pool(name="tmp", bufs=4) as tmps,
            tc.tile_pool(name="ev", bufs=4) as evs,
            tc.tile_pool(name="ps", bufs=8, space="PSUM") as ps1,
        ):
            ps2 = ps1
            # xg first: it gates the very first matmul. The other resident
            # tensors are needed only by later phases — their DMAs are
            # issued mid-way through the GEMM1 loop (side_loads) so the
            # startup slab stream gets the full HBM bandwidth.
            xg_sb = res.tile([128, DTILES, C], DT_BF16)
            for d in range(DTILES):
                nc.sync.dma_start(xg_sb[:, d, :], xg[:, d, :])

            # PE warm-up: the HAM clock gate needs ~3.4us of sustained
            # activity to lift the PE from 1.2 to 2.4 GHz. Run dummy
            # matmuls on a zeroed tile while the first DMAs land so the
            # real matmuls start warm.
            warm = res.tile([128, 512], DT_BF16)
            nc.vector.memset(warm[:], 0.0)
            pw = ps1.tile([128, 512], DT_F32, tag="ps", name="pw")
            for i in range(16):
                nc.tensor.matmul(pw[:], warm[:, :128], warm[:],
                                 start=(i == 0), stop=(i == 15))
            xt_sb = res.tile([128, DTILES, NTOK], DT_BF16)
            w2_sb = res.tile([128, MI, D], DT_BF16)
            w2s_sb = res.tile([128, MS, D], DT_BF16)
            h_sb = res.tile([128, MI, C], DT_BF16)
            hs_sb = res.tile([128, MS, NTOK], DT_BF16)

            def gemm1(npairs, wsrc, x_sb, tchunks, hout, side_loads={}):
                # hout[i, t] = silu(min(W1.T x, L)) * clip(W3.T x, -L, L)
                for m in range(npairs):
                    for fn in side_loads.get(m, []):
                        fn()
                    sg = slabs.tile([128, DTILES, 128], DT_BF16, tag="slab")
                    nc.sync.dma_start(sg[:], wsrc[2 * m])
                    su = slabs.tile([128, DTILES, 128], DT_BF16, tag="slab")
                    nc.sync.dma_start(su[:], wsrc[2 * m + 1])
                    for (t0, tl) in tchunks:
                        pg = ps1.tile([128, 512], DT_F32, tag="ps", name="pg")[:, :tl]
                        pu = ps1.tile([128, 512], DT_F32, tag="ps", name="pu")[:, :tl]
                        for d in range(DTILES):
                            nc.tensor.matmul(
                                pg[:], sg[:, d, :], x_sb[:, d, t0:t0 + tl],
                                start=(d == 0), stop=(d == DTILES - 1))
                        for d in range(DTILES):
                            nc.tensor.matmul(
                                pu[:], su[:, d, :], x_sb[:, d, t0:t0 + tl],
                                start=(d == 0), stop=(d == DTILES - 1))
                        tg = tmps.tile([128, 512], DT_F32, tag="tg", name="tg")[:, :tl]
                        nc.vector.tensor_scalar(tg[:], pg[:], LIMIT, None, MIN)
                        sa = tmps.tile([128, 512], DT_F32, tag="sa", name="sa")[:, :tl]
                        nc.scalar.activation(sa[:], tg[:], SILU)
                        tu = tmps.tile([128, 512], DT_F32, tag="tu", name="tu")[:, :tl]
                        nc.vector.tensor_scalar(tu[:], pu[:], LIMIT, -LIMIT, MIN, MAX)
                        nc.vector.tensor_mul(hout[:, m, t0:t0 + tl], sa[:], tu[:])

            def gemm2T_units(nitiles, h, w2sb, tlen_total, dst):
                # dst[d, t] = w2.T @ h — transposed output layout; PE cost
                # scales with tlen_total itself, not its 128-padded tiles.
                # The combine-weight scaling happens on the host instead.
                for (t0, tl) in _chunks(tlen_total):
                    for dt_ in range(DTILES):
                        def unit(t0=t0, tl=tl, dt_=dt_):
                            ps = ps2.tile([128, 512], DT_F32, tag="ps", name="pt")[:, :tl]
                            for m in range(nitiles):
                                nc.tensor.matmul(
                                    ps[:], w2sb[:, m, dt_ * 128:(dt_ + 1) * 128],
                                    h[:, m, t0:t0 + tl],
                                    start=(m == 0), stop=(m == nitiles - 1))
                            ev = evs.tile([128, 512], DT_F32, tag="ev", name="ev")[:, :tl]
                            if dt_ % 2 == 0:
                                nc.vector.tensor_copy(ev[:], ps[:])
                            else:
                                nc.scalar.activation(ev[:], ps[:], COPY)
                            nc.sync.dma_start(
                                dst[dt_ * 128:(dt_ + 1) * 128, t0:t0 + tl], ev[:])
                        yield unit

            def gemm2_units(nitiles, h, w2sb, tlen_total, dst):
                # dst[t, d] = h.T @ w2
                for tt, (t0, tl) in enumerate(_chunks(tlen_total, 128)):
                    for di, (d0, dl) in enumerate(DC):
                        def unit(t0=t0, tl=tl, di=di, d0=d0, dl=dl):
                            ps = ps2.tile([128, 512], DT_F32, tag="ps", name="po")[:tl, :dl]
                            for m in range(nitiles):
                                nc.tensor.matmul(
                                    ps[:], h[:, m, t0:t0 + tl],
                                    w2sb[:, m, d0:d0 + dl],
                                    start=(m == 0), stop=(m == nitiles - 1))
                            ev = evs.tile([128, 512], DT_F32, tag="ev", name="ev")[:tl, :dl]
                            if di % 2 == 0:
                                nc.vector.tensor_copy(ev[:], ps[:])
                            else:
                                nc.scalar.activation(ev[:], ps[:], COPY)
                            nc.sync.dma_start(dst[t0:t0 + tl, d0:d0 + dl], ev[:])
                        yield unit

            # Interleave the later-phase resident loads into the slab DMA
            # FIFO in small chunks so they never starve the slab stream.
            side = {}
            for j, d in enumerate(range(DTILES)):
                side.setdefault(2 + 2 * j, []).append(
                    lambda d=d: nc.sync.dma_start(xt_sb[:, d, :], xt[:, d, :]))
            for j in range(8):
                side.setdefault(14 + j, []).append(
                    lambda j=j: nc.sync.dma_start(
                        w2_sb[:, 3 * j:3 * (j + 1), :], w2[:, 3 * j:3 * (j + 1), :]))
            side.setdefault(23, []).append(
                lambda: nc.sync.dma_start(w2s_sb[:], w2s[:]))
            gemm1(MI, w13, xg_sb, TCR, h_sb, side)
            gemm1(MS, w13s, xt_sb, TCS, hs_sb)
            # Interleave the evict-heavy shared GEMM2 (many small psum
            # groups) with the evict-light routed GEMM2 (long psum
            # accumulations) so the eviction pipeline drains while PE is
            # still busy, and the kernel ends on an evict-light unit.
            r_units = list(gemm2T_units(MI, h_sb, w2_sb, C, out_r))
            # tiny tail chunks (t-remainder) last: their evictions drain fast
            r_units.sort(key=lambda u: u.__defaults__[0])
            s_units = list(gemm2_units(MS, hs_sb, w2s_sb, NTOK, out_s))
            ns, nr = len(s_units), len(r_units)
            si = 0
            for ri, ru in enumerate(r_units):
                take = (ns * (ri + 1)) // nr
                while si < min(take, ns):
                    s_units[si]()
                    si += 1
                ru()
            while si < ns:
                s_units[si]()
                si += 1

    nc.compile()
    return nc


def _slabify(w):
    """[768, ncols] -> [ncols//128, 128, 6, 128] stationary slabs.

    slab[m, p, a, f] = w[a*128 + p, m*128 + f]
    """
    ncols = w.shape[1]
    return np.ascontiguousarray(
        w.reshape(DTILES, 128, ncols // 128, 128).transpose(2, 1, 0, 3))


def _ptile(a):
    """[R, cols] with R = n*128 -> [128, n, cols] (partition-major)."""
    r, c = a.shape
    return np.ascontiguousarray(a.reshape(r // 128, 128, c).transpose(1, 0, 2))


def kernel(**inputs) -> np.ndarray:
    global last_results
    x = np.asarray(inputs["x"], dtype=np.float32)
    gate_w = np.asarray(inputs["gate_w"], dtype=np.float32)
    gate_bias = np.asarray(inputs["gate_bias"], dtype=np.float32)
    w1 = np.asarray(inputs["w1"], dtype=np.float32)
    w2 = np.asarray(inputs["w2"], dtype=np.float32)
    w3 = np.asarray(inputs["w3"], dtype=np.float32)
    w1s = np.asarray(inputs["w1s"], dtype=np.float32)
    w2s = np.asarray(inputs["w2s"], dtype=np.float32)
    w3s = np.asarray(inputs["w3s"], dtype=np.float32)

    B, T, _ = x.shape
    N = B * T
    assert N == NTOK, f"kernel compiled for {NTOK} tokens, got {N}"
    flat = x.reshape(N, D)

    # ---- gate (host, f32, mirrors reference semantics) ----
    logits = flat @ gate_w                              # [N, E]
    scores = np.sqrt(np.logaddexp(np.float32(0.0), logits)).astype(np.float32)
    routed = scores + gate_bias
    idx = np.argsort(-routed, axis=1, kind="stable")[:, :TOPK]      # [N, K]
    wts = np.take_along_axis(scores, idx, axis=1)
    wts = wts / np.clip(wts.sum(axis=1, keepdims=True), 1e-6, None)

    # ---- dispatch: per-expert token lists ----
    ee = idx.reshape(-1)
    tok = np.repeat(np.arange(N), TOPK)
    ww = wts.reshape(-1).astype(np.float32)
    toks, cwts, counts = [], [], []
    for e in range(E):
        sel = ee == e
        toks.append(tok[sel])
        cwts.append(ww[sel])
        counts.append(int(sel.sum()))
    C = max(128, ((max(counts) + 31) // 32) * 32)

    # ---- per-core input maps ----
    xt_h = _ptile(flat.T.astype(BF16))                  # [128, 6, N]
    in_maps = []
    for e in range(E):
        ce = counts[e]
        xg_full = np.zeros((C, D), dtype=np.float32)
        xg_full[:ce] = flat[toks[e]]
        cpad = ((C + 127) // 128) * 128
        cw_full = np.zeros(cpad, dtype=np.float32)
        cw_full[:ce] = cwts[e]

        w13 = np.empty((2 * MI, 128, DTILES, 128), dtype=BF16)
        w13[0::2] = _slabify(w1[e].astype(BF16))
        w13[1::2] = _slabify(w3[e].astype(BF16))
        sl = slice(e * ISH, (e + 1) * ISH)
        w13s = np.empty((2 * MS, 128, DTILES, 128), dtype=BF16)
        w13s[0::2] = _slabify(w1s[:, sl].astype(BF16))
        w13s[1::2] = _slabify(w3s[:, sl].astype(BF16))

        in_maps.append({
            "w13": w13,
            "w2": _ptile(w2[e].astype(BF16)),           # [128, 24, 768]
            "w13s": w13s,
            "w2s": _ptile(w2s[sl].astype(BF16)),        # [128, 3, 768]
            "xt": xt_h,
            "xg": _ptile(xg_full.T.astype(BF16)),       # [128, 6, C]
            "cw": np.ascontiguousarray(
                cw_full.reshape(-1, 128).T),            # [128, ceil(C/128)]
        })

    # ---- build + run ----
    if C not in _BUILD_CACHE:
        _BUILD_CACHE[C] = _build(C)
    nc = _BUILD_CACHE[C]
    last_results = run_bass_kernel_spmd(nc, in_maps, core_ids=list(range(NCORES)))
    res = last_results.results

    # ---- combine (host): sum shared partials, scatter routed outputs ----
    out = res[0]["out_s"].astype(np.float32).copy()
    for c in range(1, NCORES):
        out += res[c]["out_s"]
    for e in range(E):
        ce = counts[e]
        if ce:
            out[toks[e]] += res[e]["out_r"][:, :ce].T * cwts[e][:, None]
    return out.reshape(B, T, D).astype(np.float32)


# revision 21
# speedup vs baseline: 1.3124x; 1.0676x over previous
"""DeepSeekMoE forward on 8 TRN2 NeuronCores.

Strategy (expert-parallel, per the sharding hint):
  - Host computes the (tiny) gate: scores = sqrt(softplus(x @ gate_w)),
    top-2 selection, normalized combine weights, and builds per-expert
    token lists (the "all-to-all dispatch" done host-side since kernel()
    receives full inputs and returns the full output).
  - Core e holds routed expert e's weights and processes the tokens
    routed to it (padded to a common capacity C).
  - The shared expert is split along its intermediate dim I across the
    8 cores (each core computes a 384-wide slice for ALL tokens); the
    partial outputs sum to the exact shared-expert output.
  - Host scatters/sums the per-core outputs back to [B, T, D].

Device compute is bf16 (f32 PSUM accumulation): TRN2 PE does bf16 at
1 cycle/row vs 4 for fp32, and bf16 halves the HBM traffic.
"""

import math

import numpy as np
import ml_dtypes

import concourse.bass as bass
import concourse.tile as tile
from concourse import bacc, mybir
from concourse.bass_utils import run_bass_kernel_spmd

BF16 = np.dtype(ml_dtypes.bfloat16)
DT_BF16 = mybir.dt.bfloat16
DT_F32 = mybir.dt.float32

D = 768            # n_embd
I = 3072           # moe_intermediate_size
E = 8              # n_routed_experts
TOPK = 2
LIMIT = 10.0
NTOK = 2048        # B*T
NCORES = 8
ISH = I // NCORES  # shared-expert I slice per core (384)
DTILES = D // 128  # 6
MI = I // 128      # 24 routed i-tiles
MS = ISH // 128    # 3 shared i-tiles

_BUILD_CACHE: dict = {}
last_results = None  # BassKernelResults of the most recent run (for test.py)


def _chunks(total, step=512):
    # Balanced chunking: a trailing sliver (e.g. 32 wide) makes its
    # matmuls LDWEIGHTS-bound; equal chunks keep every matmul long
    # enough (>= ~128 rows) to hide the stationary loads.
    import math as _m
    n = max(1, _m.ceil(total / step))
    base = total // n
    rem = total - base * n
    out, t0 = [], 0
    for i in range(n):
        ln = base + (1 if i < rem else 0)
        out.append((t0, ln))
        t0 += ln
    return out


def _build(C):
    """Build the SPMD Bass graph for capacity C (tokens per routed expert)."""
    nc = bacc.Bacc("TRN2", target_bir_lowering=False, debug=False)

    ap = lambda name, shape, dt, kind: nc.dram_tensor(name, shape, dt, kind=kind).ap()
    w13 = ap("w13", [2 * MI, 128, DTILES, 128], DT_BF16, "ExternalInput")
    w2 = ap("w2", [128, MI, D], DT_BF16, "ExternalInput")
    w13s = ap("w13s", [2 * MS, 128, DTILES, 128], DT_BF16, "ExternalInput")
    w2s = ap("w2s", [128, MS, D], DT_BF16, "ExternalInput")
    xt = ap("xt", [128, DTILES, NTOK], DT_BF16, "ExternalInput")
    xg = ap("xg", [128, DTILES, C], DT_BF16, "ExternalInput")
    cw = ap("cw", [128, (C + 127) // 128], DT_F32, "ExternalInput")
    out_r = ap("out_r", [D, C], DT_F32, "ExternalOutput")
    out_s = ap("out_s", [NTOK, D], DT_F32, "ExternalOutput")

    TCR = _chunks(C)      # routed token chunks
    TCS = _chunks(NTOK)   # shared token chunks
    DC = _chunks(D)       # output d chunks (512, 256)

    MIN = mybir.AluOpType.min
    MAX = mybir.AluOpType.max
    SILU = mybir.ActivationFunctionType.Silu
    COPY = mybir.ActivationFunctionType.Copy

    with tile.TileContext(nc) as tc:
        with (
            tc.tile_pool(name="res", bufs=1) as res,
            tc.tile_pool(name="slab", bufs=8) as slabs,
            tc.tile_pool(name="tmp", bufs=4) as tmps,
            tc.tile_pool(name="ev", bufs=4) as evs,
            tc.tile_pool(name="ps", bufs=8, space="PSUM") as ps1,
        ):
            ps2 = ps1
            # xg first: it gates the very first matmul. The other resident
            # tensors are needed only by later phases — their DMAs are
            # issued mid-way through the GEMM1 loop (side_loads) so the
            # startup slab stream gets the full HBM bandwidth.
            xg_sb = res.tile([128, DTILES, C], DT_BF16)
            for d in range(DTILES):
                nc.sync.dma_start(xg_sb[:, d, :], xg[:, d, :])

            # PE warm-up: the HAM clock gate needs ~3.4us of sustained
            # activity to lift the PE from 1.2 to 2.4 GHz. Run dummy
            # matmuls on a zeroed tile while the first DMAs land so the
            # real matmuls start warm.
            warm = res.tile([128, 512], DT_BF16)
            nc.vector.memset(warm[:], 0.0)
            pw = ps1.tile([128, 512], DT_F32, tag="ps", name="pw")
            for i in range(16):
                nc.tensor.matmul(pw[:], warm[:, :128], warm[:],
                                 start=(i == 0), stop=(i == 15))
            xt_sb = res.tile([128, DTILES, NTOK], DT_BF16)
            w2_sb = res.tile([128, MI, D], DT_BF16)
            w2s_sb = res.tile([128, MS, D], DT_BF16)
            h_sb = res.tile([128, MI, C], DT_BF16)
            hs_sb = res.tile([128, MS, NTOK], DT_BF16)

            def gemm1(npairs, wsrc, x_sb, tchunks, hout, side_loads={}):
                # hout[i, t] = silu(min(W1.T x, L)) * clip(W3.T x, -L, L)
                for m in range(npairs):
                    for fn in side_loads.get(m, []):
                        fn()
                    sg = slabs.tile([128, DTILES, 128], DT_BF16, tag="slab")
                    nc.sync.dma_start(sg[:], wsrc[2 * m])
                    su = slabs.tile([128, DTILES, 128], DT_BF16, tag="slab")
                    nc.sync.dma_start(su[:], wsrc[2 * m + 1])
                    for (t0, tl) in tchunks:
                        pg = ps1.tile([128, 512], DT_F32, tag="ps", name="pg")[:, :tl]
                        pu = ps1.tile([128, 512], DT_F32, tag="ps", name="pu")[:, :tl]
                        for d in range(DTILES):
                            nc.tensor.matmul(
                                pg[:], sg[:, d, :], x_sb[:, d, t0:t0 + tl],
                                start=(d == 0), stop=(d == DTILES - 1))
                        for d in range(DTILES):
                            nc.tensor.matmul(
                                pu[:], su[:, d, :], x_sb[:, d, t0:t0 + tl],
                                start=(d == 0), stop=(d == DTILES - 1))
                        tg = tmps.tile([128, 512], DT_F32, tag="tg", name="tg")[:, :tl]
                        nc.vector.tensor_scalar(tg[:], pg[:], LIMIT, None, MIN)
                        sa = tmps.tile([128, 512], DT_F32, tag="sa", name="sa")[:, :tl]
                        nc.scalar.activation(sa[:], tg[:], SILU)
                        tu = tmps.tile([128, 512], DT_F32, tag="tu", name="tu")[:, :tl]
                        nc.vector.tensor_scalar(tu[:], pu[:], LIMIT, -LIMIT, MIN, MAX)
                        nc.vector.tensor_mul(hout[:, m, t0:t0 + tl], sa[:], tu[:])

            def gemm2T_units(nitiles, h, w2sb, tlen_total, dst):
                # dst[d, t] = w2.T @ h — transposed output layout; PE cost
                # scales with tlen_total itself, not its 128-padded tiles.
                # The combine-weight scaling happens on the host instead.
                for (t0, tl) in _chunks(tlen_total):
                    for dt_ in range(DTILES):
                        def unit(t0=t0, tl=tl, dt_=dt_):
                            ps = ps2.tile([128, 512], DT_F32, tag="ps", name="pt")[:, :tl]
                            for m in range(nitiles):
                                nc.tensor.matmul(
                                    ps[:], w2sb[:, m, dt_ * 128:(dt_ + 1) * 128],
                                    h[:, m, t0:t0 + tl],
                                    start=(m == 0), stop=(m == nitiles - 1))
                            ev = evs.tile([128, 512], DT_F32, tag="ev", name="ev")[:, :tl]
                            if dt_ % 2 == 0:
                                nc.vector.tensor_copy(ev[:], ps[:])
                            else:
                                nc.scalar.activation(ev[:], ps[:], COPY)
                            nc.sync.dma_start(
                                dst[dt_ * 128:(dt_ + 1) * 128, t0:t0 + tl], ev[:])
                        yield unit

            def gemm2_units(nitiles, h, w2sb, tlen_total, dst):
                # dst[t, d] = h.T @ w2
                for tt, (t0, tl) in enumerate(_chunks(tlen_total, 128)):
                    for di, (d0, dl) in enumerate(DC):
                        def unit(t0=t0, tl=tl, di=di, d0=d0, dl=dl):
                            ps = ps2.tile([128, 512], DT_F32, tag="ps", name="po")[:tl, :dl]
                            for m in range(nitiles):
                                nc.tensor.matmul(
                                    ps[:], h[:, m, t0:t0 + tl],
                                    w2sb[:, m, d0:d0 + dl],
                                    start=(m == 0), stop=(m == nitiles - 1))
                            ev = evs.tile([128, 512], DT_F32, tag="ev", name="ev")[:tl, :dl]
                            if di % 2 == 0:
                                nc.vector.tensor_copy(ev[:], ps[:])
                            else:
                                nc.scalar.activation(ev[:], ps[:], COPY)
                            nc.sync.dma_start(dst[t0:t0 + tl, d0:d0 + dl], ev[:])
                        yield unit

            # Interleave the later-phase resident loads into the slab DMA
            # FIFO in small chunks so they never starve the slab stream.
            side = {}
            for j, d in enumerate(range(DTILES)):
                side.setdefault(2 + 2 * j, []).append(
                    lambda d=d: nc.sync.dma_start(xt_sb[:, d, :], xt[:, d, :]))
            for j in range(8):
                side.setdefault(14 + j, []).append(
                    lambda j=j: nc.sync.dma_start(
                        w2_sb[:, 3 * j:3 * (j + 1), :], w2[:, 3 * j:3 * (j + 1), :]))
            side.setdefault(23, []).append(
                lambda: nc.sync.dma_start(w2s_sb[:], w2s[:]))
            gemm1(MI, w13, xg_sb, TCR, h_sb, side)
            gemm1(MS, w13s, xt_sb, TCS, hs_sb)
            # Interleave the evict-heavy shared GEMM2 (many small psum
            # groups) with the evict-light routed GEMM2 (long psum
            # accumulations) so the eviction pipeline drains while PE is
            # still busy, and the kernel ends on an evict-light unit.
            r_units = list(gemm2T_units(MI, h_sb, w2_sb, C, out_r))
            # tiny tail chunks (t-remainder) last: their evictions drain fast
            r_units.sort(key=lambda u: u.__defaults__[0])
            s_units = list(gemm2_units(MS, hs_sb, w2s_sb, NTOK, out_s))
            ns, nr = len(s_units), len(r_units)
            si = 0
            for ri, ru in enumerate(r_units):
                take = (ns * (ri + 1)) // nr
                while si < min(take, ns):
                    s_units[si]()
                    si += 1
                ru()
            while si < ns:
                s_units[si]()
                si += 1

    nc.compile()
    return nc


def _slabify(w):
    """[768, ncols] -> [ncols//128, 128, 6, 128] stationary slabs.

    slab[m, p, a, f] = w[a*128 + p, m*128 + f]
    """
    ncols = w.shape[1]
    return np.ascontiguousarray(
        w.reshape(DTILES, 128, ncols // 128, 128).transpose(2, 1, 0, 3))


def _ptile(a):
    """[R, cols] with R = n*128 -> [128, n, cols] (partition-major)."""
    r, c = a.shape
    return np.ascontiguousarray(a.reshape(r // 128, 128, c).transpose(1, 0, 2))


def kernel(**inputs) -> np.ndarray:
    global last_results
    x = np.asarray(inputs["x"], dtype=np.float32)
    gate_w = np.asarray(inputs["gate_w"], dtype=np.float32)
    gate_bias = np.asarray(inputs["gate_bias"], dtype=np.float32)
    w1 = np.asarray(inputs["w1"], dtype=np.float32)
    w2 = np.asarray(inputs["w2"], dtype=np.float32)
    w3 = np.asarray(inputs["w3"], dtype=np.float32)
    w1s = np.asarray(inputs["w1s"], dtype=np.float32)
    w2s = np.asarray(inputs["w2s"], dtype=np.float32)
    w3s = np.asarray(inputs["w3s"], dtype=np.float32)

    B, T, _ = x.shape
    N = B * T
    assert N == NTOK, f"kernel compiled for {NTOK} tokens, got {N}"
    flat = x.reshape(N, D)

    # ---- gate (host, f32, mirrors reference semantics) ----
    logits = flat @ gate_w                              # [N, E]
    scores = np.sqrt(np.logaddexp(np.float32(0.0), logits)).astype(np.float32)
    routed = scores + gate_bias
    idx = np.argsort(-routed, axis=1, kind="stable")[:, :TOPK]      # [N, K]
    wts = np.take_along_axis(scores, idx, axis=1)
    wts = wts / np.clip(wts.sum(axis=1, keepdims=True), 1e-6, None)

    # ---- dispatch: per-expert token lists ----
    ee = idx.reshape(-1)
    tok = np.repeat(np.arange(N), TOPK)
    ww = wts.reshape(-1).astype(np.float32)
    toks, cwts, counts = [], [], []
    for e in range(E):
        sel = ee == e
        toks.append(tok[sel])
        cwts.append(ww[sel])
        counts.append(int(sel.sum()))
    C = max(128, ((max(counts) + 31) // 32) * 32)

    # ---- per-core input maps ----
    xt_h = _ptile(flat.T.astype(BF16))                  # [128, 6, N]
    in_maps = []
    for e in range(E):
        ce = counts[e]
        xg_full = np.zeros((C, D), dtype=np.float32)
        xg_full[:ce] = flat[toks[e]]
        cpad = ((C + 127) // 128) * 128
        cw_full = np.zeros(cpad, dtype=np.float32)
        cw_full[:ce] = cwts[e]

        w13 = np.empty((2 * MI, 128, DTILES, 128), dtype=BF16)
        w13[0::2] = _slabify(w1[e].astype(BF16))
        w13[1::2] = _slabify(w3[e].astype(BF16))
        sl = slice(e * ISH, (e + 1) * ISH)
        w13s = np.empty((2 * MS, 128, DTILES, 128), dtype=BF16)
        w13s[0::2] = _slabify(w1s[:, sl].astype(BF16))
        w13s[1::2] = _slabify(w3s[:, sl].astype(BF16))

        in_maps.append({
            "w13": w13,
            "w2": _ptile(w2[e].astype(BF16)),           # [128, 24, 768]
            "w13s": w13s,
            "w2s": _ptile(w2s[sl].astype(BF16)),        # [128, 3, 768]
            "xt": xt_h,
            "xg": _ptile(xg_full.T.astype(BF16)),       # [128, 6, C]
            "cw": np.ascontiguousarray(
                cw_full.reshape(-1, 128).T),            # [128, ceil(C/128)]
        })

    # ---- build + run ----
    if C not in _BUILD_CACHE:
        _BUILD_CACHE[C] = _build(C)
    nc = _BUILD_CACHE[C]
    last_results = run_bass_kernel_spmd(nc, in_maps, core_ids=list(range(NCORES)))
    res = last_results.results

    # ---- combine (host): sum shared partials, scatter routed outputs ----
    out = res[0]["out_s"].astype(np.float32).copy()
    for c in range(1, NCORES):
        out += res[c]["out_s"]
    for e in range(E):
        ce = counts[e]
        if ce:
            out[toks[e]] += res[e]["out_r"][:, :ce].T * cwts[e][:, None]
    return out.reshape(B, T, D).astype(np.float32)
